# revision 41
# baseline (speedup 1.0000x reference)
"""Causal multi-head attention on 8 Trainium2 NeuronCores (Bass/Tile).

Problem: Q,K,V [B=2, h=16, S=2048, d=64] fp32; out = softmax(QK^T/8, causal) V.

Sharding: B*h = 32 heads split 4-per-core across 8 cores (head-parallel);
each core computes full causal attention for its 4 heads.

Schedule (vs. the 185us baseline): the PE program is software-pipelined
with skew 2 over a flat global (head, chunk, pair) list so the PE never
waits on softmax: ..., QK(i), PV(i-2), QK(i+1), PV(i-1), ... Keeping the PE
continuously busy also keeps it at the 2.4GHz pstate (an idle PE throttles
to 1.2GHz, which is where most of the baseline's time went).

Engine split: the softmax exp is COLUMN-SPLIT between ACT (exact exp,
q-columns [EXP_SPLIT:512) of each half) and DVE (Schraudolph fast-exp,
columns [crop:EXP_SPLIT)) — the two engines run in parallel on every
pair, which halves the exp leg of the QK->exp->PV chain and splits the
~80us/core of exp work across both engines (ACT alone was an 86us
bottleneck). A given q-column uses ONE engine for its entire softmax
row, so the fexp bias cancels in p/l; measured absmax-rel err 1.2e-2
(gate 2e-2). Diagonal-block causal masking runs on GPSIMD as a direct
affine_select on P^T (GPSIMD cannot touch PSUM, but P^T is SBUF). DVE
keeps the PSUM->SBUF copies + output normalize. GPSIMD issues the
(casting) input DMAs, staggered at pair slots 2/3/6 so the ~1.4us SWDGE
issue instructions never queue ahead of the masks; prep transposes run
at slots 8-13. SP issues Q^T row-dup + batched output stores.

NOTE: do NOT emit pe_touch instructions that wait on in-flight DMAs in
the middle of the pair stream — the in-order PE queue stalls on them
(and detached touches produced nondeterministically wrong results);
split_waits handles multi-wait instructions instead.

QK side runs bf16 (SWDGE casting loads, PE transpose-mode); the P/V side
runs bf16: exp writes bf16 P^T directly, V' = [V | 1] is DMA-cast to bf16,
so the PV matmuls take 1 cycle/row at any crop width (fp32r would pay 4x
below 256-wide). The two QK matmuls of a pair occupy row groups h0/h64
(64-contraction each) and run CONCURRENTLY in the PE array.

Known dead ends (measured): whole-pair alternation of exp between
ACT/DVE leaves the per-pair latency chain at ~1.1us and paces the whole
kernel; merging exp instructions across pairs via one persistent
PSUM/SBUF ring tensor serializes the pipeline (Tile's overlap tracking
over a single big tile is conservative: TimelineSim 169us vs 102us);
pool-based exp merge (double-wide [128,2048] st tiles, one tile per
merged exp so per-tile deps are exact) is PSUM-capacity-blocked: the
double tile needs bufs=2 = 8 banks + ot + stage > 8, and bufs=1 makes
the next QK group wait out the full ~1.4us merged ACT exp (TimelineSim
136us); EXP_SPLIT=200 overloads DVE and regresses to 123.1us (HW);
SKEW=3 (legal with st bufs=3 / pt bufs=4) regresses to 120.0us (HW);
36 seam-filler dummy matmuls before head-1's first QK (to keep HAM warm
through its ~4us row-dup wait) regress to 117.8us — the post-seam region
is exp-chain-paced, so PE warmth does not pay for the filler time;
XBAR DMA-transpose cannot produce the [d, 128t+p] Q^T layout (AP
walk-order mismatch); PV col-tiling needs 65+65 > 128 output partitions.
Best measured: 117711 ns (this file), rel err 1.214e-2, vs 133888 ns
session baseline.

Per-head layout:
  - Q,K loaded [128, 16*64] fp32->fp32r via SWDGE; V' [128, 16*65] bf16.
  - PE transpose-mode: Q -> Q^T [64, 2048] fp32r (+SP DMA row-dup to
    64:128), K -> K^T stacked pairs [128, 8*128] fp32r; PSUM->SBUF copies
    on DVE.
  - Pair (c, t): S^T [128, 1024] PSUM (two 64-contraction matmuls, min-256
    crops), ACT exp -> P^T bf16, diag mask, PV: O'^T [65, 512] += V'_j^T @
    P^T_j (row 64 = softmax denominator l).
  - Per chunk: O'^T -> SBUF bf16 (DVE), PE transpose to [128, 4*96] PSUM,
    one batched reciprocal + one broadcast multiply (DVE), one batched
    output store (SP).
"""

import numpy as np

import concourse.bass as bass
import concourse.bass_utils as _bass_utils
import concourse.mybir as mybir
import concourse.tile as tile
from concourse.bass_utils import run_bass_kernel_spmd
from concourse.tile import add_dep_helper

# NOTE: walrus's --enable-ldw-opt=true was tried to dedup/overlap the
# per-matmul LDWEIGHTS (~53us/core serial) but miscompiles this kernel
# (NaN output) — the flag stays at the default false.

N_CORES = 8
B, H, S, D = 2, 16, 2048, 64
HEADS_PER_CORE = (B * H) // N_CORES  # 4
NT = S // 128           # 16 k/q tiles per head
NCHUNK = S // 512       # 4 q-chunks per head
PAIRS_PER_HEAD = sum(2 * c + 2 for c in range(NCHUNK))  # 20
F32 = mybir.dt.float32
F32R = mybir.dt.float32r
BF16 = mybir.dt.bfloat16
I16 = mybir.dt.int16

# Schraudolph fast-exp (bf16 domain): exp(0.125*s) ~= bitcast_bf16(int16(
# s*K1 + K2)). Used only for pairs selected by FEXP_PATTERN.
FEXP_K1 = float(np.float32(0.125 * 1.4426950408889634 * 128))
FEXP_K2 = float(np.float32((127.0 - 0.04367744) * 128))
# Column-split exp: every pair's softmax exp is split by q-columns between
# ACT (exact exp, columns [SPLIT:512] of each 512-half) and DVE (Schraudolph
# fast-exp, columns [crop:SPLIT]). Per-q-column consistency: a given q uses
# one engine for ALL its k across the whole row, so the fexp bias cancels in
# p/l and the net error stays ~1e-3.
EXP_SPLIT = 184  # DVE takes [a:184) of each half; ACT takes [184:512)


class SplitDrainTileContext(tile.TileContext):
    """TileContext whose tail drain splits its semaphore waits across
    single-wait SP nops — the TPB CTRL_NO struct holds one wait slot, so
    a drain waiting on >1 proc fails walrus codegen."""

    def _drain_and_barrier(self, tick_clock, wait_clock):
        import bass_rust
        from concourse.vector_clock import ScopedClock

        gc = tick_clock.global_clock
        for i, v in enumerate(list(gc)):
            if v <= 0:
                continue
            c = bass_rust.VectorClock()
            c.require_at_least(i, v)
            nop = self.nc.sync.nop(hint="preDrain", nofuse=True)
            wait_clock.add_sem_waits(nop.ins, ScopedClock({None: c}))
        drain_inst = self.nc.sync.drain()
        wait_clock.add_sem_waits(
            drain_inst.ins, ScopedClock({None: bass_rust.VectorClock()})
        )
        self.nc.all_engine_barrier()
        assert self.sems is not None
        popped = self.nc._tile_sem_poison_stack.pop()
        assert popped is self._sem_poison
        self.nc.clear_and_free_semaphores(list(self.sems.allocated().values()))
        self.nc.all_engine_barrier()


def pe_touch(nc, ap):
    """1-column bf16 ldweights reading `ap` — engine-level PE instruction
    that absorbs a producer's sync wait into the PE engine clock so that
    following 4-byte matmuls need at most one wait (walrus S3_LW limit)."""
    return nc.tensor.ldweights(ap.bitcast(mybir.dt.bfloat16))


def split_waits(nc):
    """Post-pass: every TPB instruction holds exactly ONE sync-wait slot;
    walrus codegen rejects more. Move extra waits onto inserted same-engine
    nofuse nops placed immediately before the instruction."""
    cnt = 0
    for fn in nc.m.functions:
        for bb in fn.blocks:
            lst = bb.instructions
            i = 0
            while i < len(lst):
                ins = lst[i]
                si = ins.sync_info
                if si is not None and si.on_wait and len(si.on_wait) > 1:
                    waits = list(si.on_wait)
                    for w in waits[:-1]:
                        nop = mybir.InstNoOp(name=f"wsplit_{cnt}", ins=[], outs=[])
                        cnt += 1
                        nop.engine = ins.engine
                        nop.bass_nofuse = True
                        nop.sync_info = mybir.SyncInfo(on_wait=[w], on_update=[])
                        lst.insert(i, nop)
                        i += 1
                    si.on_wait = [waits[-1]]
                i += 1
    return cnt


def build_kernel():
    nc = bass.Bass(trn_type="TRN2")
    q_d = nc.dram_tensor("Q", [HEADS_PER_CORE, S, D], F32, kind="ExternalInput")
    k_d = nc.dram_tensor("K", [HEADS_PER_CORE, S, D], F32, kind="ExternalInput")
    v_d = nc.dram_tensor("V", [HEADS_PER_CORE, S, D], F32, kind="ExternalInput")
    o_d = nc.dram_tensor("O", [HEADS_PER_CORE, S, D], F32, kind="ExternalOutput")

    with SplitDrainTileContext(nc) as tc:
        import contextlib

        with contextlib.ExitStack() as ctx:
            consts = ctx.enter_context(tc.tile_pool(name="consts", bufs=1))
            in_pool = ctx.enter_context(tc.tile_pool(name="in", bufs=2))
            v_pool = ctx.enter_context(tc.tile_pool(name="vp", bufs=2))
            qt_pool = ctx.enter_context(tc.tile_pool(name="qt", bufs=2))
            kt_pool = ctx.enter_context(tc.tile_pool(name="kt", bufs=2))
            pt_pool = ctx.enter_context(tc.tile_pool(name="pt", bufs=4))
            otsb_pool = ctx.enter_context(tc.tile_pool(name="otsb", bufs=2))
            ob_pool = ctx.enter_context(tc.tile_pool(name="ob", bufs=2))
            r_pool = ctx.enter_context(tc.tile_pool(name="recip", bufs=4))

            st_ps = ctx.enter_context(tc.tile_pool(name="stps", bufs=3, space="PSUM"))
            ot_ps = ctx.enter_context(tc.tile_pool(name="otps", bufs=1, space="PSUM"))
            stage_ps = ctx.enter_context(tc.tile_pool(name="stage", bufs=1, space="PSUM"))

            # ---- constants ----
            ident_f = consts.tile([128, 128], F32, tag="ident_f")
            nc.gpsimd.memset(ident_f[:], 0.0)
            nc.gpsimd.affine_select(
                out=ident_f[:], in_=ident_f[:],
                compare_op=mybir.AluOpType.not_equal, fill=1.0, base=0,
                pattern=[[-1, 128]], channel_multiplier=1,
            )
            ident_r = consts.tile([128, 128], F32R, tag="ident_r")
            nc.vector.tensor_copy(ident_r[:], ident_f[:])
            ident_b = consts.tile([128, 128], BF16, tag="ident_b")
            nc.vector.tensor_copy(ident_b[:], ident_f[:])
            # 0/1 causal keep-mask for one diagonal block of P^T [k, q]:
            # keep (1.0) where q >= k i.e. f >= p, zero where f < p.
            tmask = consts.tile([128, 128], BF16, tag="tmask")
            nc.gpsimd.memset(tmask[:], 1.0)
            # keep 1.0 where f - p + 1 > 0 i.e. q >= k; fill 0.0 above diag
            nc.gpsimd.affine_select(
                out=tmask[:], in_=tmask[:],
                compare_op=mybir.AluOpType.is_gt, fill=0.0, base=1,
                pattern=[[1, 128]], channel_multiplier=-1,
            )
            t_if = pe_touch(nc, ident_f[0:1, 0:1])
            t_ir = pe_touch(nc, ident_r[0:1, 0:1])
            t_ib = pe_touch(nc, ident_b[0:1, 0:1])
            # PE warm-up: keep the array busy early so the pstate ramps to
            # full clock while the first loads land.
            warm = stage_ps.tile([128, 512], F32, tag="stage")
            for _ in range(36):
                nc.tensor.matmul(
                    warm[:, 0:256],
                    ident_f[:, 0:64].bitcast(mybir.dt.bfloat16),
                    ident_f[:, 0:128].bitcast(mybir.dt.bfloat16),
                    start=True, stop=True,
                )

            # ---- per-head prep pieces ----
            # Loads are staggered across pair slots (k, then q, then v) so
            # the ~1.4us SWDGE issue instructions on GPSIMD never queue up
            # in front of the diagonal masks, and the PE touches (which
            # carry the DMA-completion waits) are emitted only once the
            # loads have had several pair-slots to land — emitting them
            # with the loads stalls the in-order PE queue for the full DMA
            # latency at every head boundary.
            def emit_load_k(h):
                qn = in_pool.tile([128, NT * 64], BF16, tag="qn")
                kn = in_pool.tile([128, NT * 64], BF16, tag="kn")
                nc.gpsimd.dma_start(
                    kn[:].rearrange("p (t d) -> p t d", d=64),
                    k_d[h].rearrange("(t p) d -> p t d", p=128),
                )
                qt = qt_pool.tile([128, S], BF16, tag="qt")
                kt = kt_pool.tile([128, 8 * 128], BF16, tag="kt")
                return {"qn": qn, "kn": kn, "qt": qt, "kt": kt,
                        "first_tr": None}

            def emit_load_q(hs, h):
                nc.gpsimd.dma_start(
                    hs["qn"][:].rearrange("p (t d) -> p t d", d=64),
                    q_d[h].rearrange("(t p) d -> p t d", p=128),
                )

            def emit_load_v(hs, h):
                vp = v_pool.tile([128, NT * 65], BF16, tag="vp")
                vp3 = vp[:].rearrange("p (t e) -> p t e", e=65)
                nc.gpsimd.dma_start(
                    vp3[:, :, 0:64],
                    v_d[h].rearrange("(t p) d -> p t d", p=128),
                )
                nc.gpsimd.memset(vp3[:, :, 64:65], 1.0)
                hs["vp"] = vp

            def emit_loads(h):
                hs = emit_load_k(h)
                emit_load_q(hs, h)
                emit_load_v(hs, h)
                hs["touch"] = []
                return hs

            def emit_group(hs, g):
                """g 0..3: Q transpose groups, bf16 PE transpose-mode
                (DVE copy + SP row-dup); g 4..5: K^T stacked pairs, bf16
                PE transpose-mode (DVE copy). An XBAR DMA-transpose K was
                tried: each occupies the Sync engine ~1.2us and starves
                the PE at head boundaries — PE transposes are cheaper."""
                if g < 4:
                    stage = stage_ps.tile([128, 512], BF16, tag="stage",
                                          name="stage")
                    for s_i in range(4):
                        b = 4 * g + s_i
                        mm = nc.tensor.transpose(
                            stage[0:64, 128 * s_i:128 * s_i + 128],
                            hs["qn"][:, 64 * b:64 * b + 64],
                            ident_b[0:128, 0:128],
                        )
                        if hs["first_tr"] is None:
                            hs["first_tr"] = mm
                            for t in [t_if, t_ir, t_ib] + hs["touch"]:
                                if t is not None:
                                    add_dep_helper(mm.ins, t.ins, sync=False,
                                                   reason="presync")
                    nc.vector.tensor_copy(
                        hs["qt"][0:64, 512 * g:512 * g + 512],
                        stage[0:64, :],
                    )
                    nc.sync.dma_start(
                        hs["qt"][64:128, 512 * g:512 * g + 512],
                        hs["qt"][0:64, 512 * g:512 * g + 512],
                    )
                else:
                    gg = g - 4
                    stage = stage_ps.tile([128, 512], BF16, tag="stage",
                                          name="stage")
                    for s_i in range(4):
                        t_i = 4 * gg + s_i
                        mm = nc.tensor.transpose(
                            stage[:, 128 * s_i:128 * s_i + 128],
                            hs["kn"][:, 128 * t_i:128 * t_i + 128],
                            ident_b[0:128, 0:128],
                        )
                        if hs["first_tr"] is None:
                            hs["first_tr"] = mm
                            for t in [t_if, t_ir, t_ib] + hs["touch"]:
                                if t is not None:
                                    add_dep_helper(mm.ins, t.ins, sync=False,
                                                   reason="presync")
                    nc.vector.tensor_copy(
                        hs["kt"][:, 512 * gg:512 * gg + 512], stage[:, :]
                    )
                if g == 3:
                    hs["tq1"] = pe_touch(nc, hs["qt"][0:1, 0:1])
                    hs["tk1"] = pe_touch(nc, hs["kt"][0:1, 0:1])

            # ---- pair ops ----
            exp_ctr = [0]

            def emit_qk(hs, h, c, t, first_of_head):
                qt, kt = hs["qt"], hs["kt"]
                j1, j2 = 2 * t, 2 * t + 1
                cA = 128 * j1 - 512 * c
                cB = 128 * j2 - 512 * c
                a1 = max(0, cA)
                a2 = max(0, cB)
                st = st_ps.tile([128, 1024], F32, tag="st")
                mmA = nc.tensor.matmul(
                    st[:, a1:512],
                    kt[0:64, 128 * t:128 * t + 128],
                    qt[0:64, 512 * c + a1:512 * c + 512],
                    start=True, stop=True,
                )
                if first_of_head:
                    for tt in (hs.get("tq1"), hs.get("tk1")):
                        if tt is not None:
                            add_dep_helper(mmA.ins, tt.ins, sync=False,
                                           reason="presync")
                nc.tensor.matmul(
                    st[:, 512 + a2:1024],
                    kt[64:128, 128 * t:128 * t + 128],
                    qt[64:128, 512 * c + a2:512 * c + 512],
                    start=True, stop=True,
                )

                pt = pt_pool.tile([128, 1024], BF16, tag="pt")
                exp_ctr[0] += 1
                # [128, 2, 512] half-major views of P^T / S^T; half 1's
                # [a1:a2) sliver is over-computed as before (harmless).
                pv2 = pt[:].rearrange("p (h x) -> p h x", x=512)
                pi2 = pt[:].bitcast(I16).rearrange("p (h x) -> p h x", x=512)
                sv2 = st[:].rearrange("p (h x) -> p h x", x=512)
                lo = a1
                if lo < EXP_SPLIT - 32:
                    nc.vector.tensor_scalar(
                        pi2[:, :, lo:EXP_SPLIT],
                        sv2[:, :, lo:EXP_SPLIT],
                        FEXP_K1, FEXP_K2,
                        mybir.AluOpType.mult, mybir.AluOpType.add,
                    )
                    m = EXP_SPLIT
                else:
                    m = lo
                nc.scalar.activation(
                    pv2[:, :, m:512], sv2[:, :, m:512],
                    mybir.ActivationFunctionType.Exp, scale=0.125,
                )
                # zero the in-block upper triangles of diagonal tiles: the
                # last two pairs of each chunk hold them, at in-pair col
                # offsets (0, 640) for pair 2c and (256, 896) for pair 2c+1.
                npair = 2 * c + 2
                if t >= npair - 2:
                    off = 0 if t == npair - 2 else 256
                    v8 = pt[:].rearrange("p (i x) -> p i x", x=128)
                    i0 = off // 128
                    dview = v8[:, i0:i0 + 6:5, :]
                    # zero where q < k i.e. f - p + 1 <= 0; runs on GPSIMD
                    # (SBUF-only engine) to keep DVE free for fast-exp.
                    nc.gpsimd.affine_select(
                        out=dview, in_=dview,
                        compare_op=mybir.AluOpType.is_gt, fill=0.0, base=1,
                        pattern=[[0, 2], [1, 128]], channel_multiplier=-1,
                    )
                return {"st": st, "pt": pt}

            def emit_pv(hs, h, c, t, tiles, ot_holder):
                pt, vp = tiles["pt"], hs["vp"]
                npair = 2 * c + 2
                if t == 0:
                    ot_holder["ot"] = ot_ps.tile([65, 512], F32, tag="ot",
                                                 name="ot")
                ot = ot_holder["ot"]
                for half, j in enumerate((2 * t, 2 * t + 1)):
                    vA = max(0, 128 * j - 512 * c)
                    nc.tensor.matmul(
                        ot[:, vA:512],
                        vp[:, 65 * j:65 * j + 65],
                        pt[:, 512 * half + vA:512 * half + 512],
                        start=(t == 0 and half == 0),
                        stop=(t == npair - 1 and half == 1),
                        skip_group_check=True,
                    )

            def emit_out_copy(ot_holder):
                ot = ot_holder["ot"]
                otsb = otsb_pool.tile([65, 512], F32R, tag="otsb")
                nc.vector.tensor_copy(otsb[:], ot[:])
                ot_holder["otsb"] = otsb

            def emit_out(hs, h, c, ot_holder):
                otsb = ot_holder["otsb"]
                oq = stage_ps.tile([128, 384], F32R, tag="stage", name="oq")
                for i in range(4):
                    nc.tensor.transpose(
                        oq[:, 96 * i:96 * i + 96],
                        otsb[0:65, 128 * i:128 * i + 128],
                        ident_r[0:65, 0:96],
                    )
                oq4 = oq[:].bitcast(F32).rearrange("p (i x) -> p i x", x=96)
                rec = r_pool.tile([128, 4], F32, tag="rec")
                nc.vector.reciprocal(rec[:][:, :, None], oq4[:, :, 64:65])
                ob = ob_pool.tile([128, 256], F32, tag="ob")
                nc.vector.tensor_tensor(
                    ob[:].rearrange("p (i x) -> p i x", x=64),
                    oq4[:, :, 0:64],
                    rec[:].broadcast_to([128, 4, 64]),
                    mybir.AluOpType.mult,
                )
                nc.sync.dma_start(
                    o_d[h, 512 * c:512 * c + 512, :].rearrange(
                        "(t p) d -> p t d", p=128),
                    ob[:].rearrange("p (t d) -> p t d", d=64),
                )

            # ---- flat skew-2 pipeline over all (head, chunk, pair) ----
            all_pairs = []
            for h in range(HEADS_PER_CORE):
                for c in range(NCHUNK):
                    for t in range(2 * c + 2):
                        all_pairs.append((h, c, t))

            # K^T groups first so kt is ready when a head's first QK fires;
            # head 0 interleaves its last Q groups into its first pairs.
            PREP_ORDER = [4, 5, 0, 1, 2, 3]
            # need-order K0, Q0, K8, Q1, Q2, Q3 spaced 2 slots apart
            GROUP_SLOTS = {8: 4, 10: 0, 12: 5, 14: 1, 16: 2, 18: 3}
            GROUP_SLOTS_H1 = {4: 4, 6: 0, 8: 5, 10: 1, 12: 2, 14: 3}
            head_state = [None] * HEADS_PER_CORE
            head_state[0] = emit_loads(0)
            # head 1's K/Q loads issue up-front: head 0's early pairs run
            # at the cold 1.2GHz clock, so slot-based prep lead is too
            # short in wall time — without this, head 1's first QKs wait
            # ~3.6us on the row-dup DMAs and re-trigger HAM cold.
            head_state[1] = emit_load_k(1)
            head_state[1]["touch"] = []
            emit_load_q(head_state[1], 1)
            for g in PREP_ORDER[:3]:
                emit_group(head_state[0], g)

            tiles_by_idx = {}
            ot_holders = {}
            out_queue = []  # (due_slot, h, c, holder): PE out-part delayed
            n = len(all_pairs)
            SKEW = 2
            OUT_DELAY = 0

            def run_pv(ip):
                hp, cp, tp = all_pairs[ip]
                key = (hp, cp)
                if key not in ot_holders:
                    ot_holders[key] = {}
                emit_pv(head_state[hp], hp, cp, tp, tiles_by_idx.pop(ip),
                        ot_holders[key])
                if tp == 2 * cp + 1:
                    holder = ot_holders.pop(key)
                    emit_out_copy(holder)
                    out_queue.append([ip + OUT_DELAY, hp, cp, holder])

            def flush_outs(slot):
                while out_queue and out_queue[0][0] <= slot:
                    _, hp, cp, holder = out_queue.pop(0)
                    emit_out(head_state[hp], hp, cp, holder)

            for i, (h, c, t) in enumerate(all_pairs):
                local = i - PAIRS_PER_HEAD * h
                tiles_by_idx[i] = emit_qk(
                    head_state[h], h, c, t, first_of_head=(local == 0))
                if h == 0 and local in (0, 2, 4):
                    emit_group(head_state[0], PREP_ORDER[3 + local // 2])
                if i >= SKEW:
                    run_pv(i - SKEW)
                    flush_outs(i - SKEW)
                if h + 1 < HEADS_PER_CORE:
                    # loads on GPSIMD avoid mask slots (0,1,4,5,10,11,18,19);
                    # groups start once kn/qn have landed so the in-order PE
                    # queue never waits long, and the row-dup DMAs finish
                    # well before the next head's first QK needs them.
                    # Head 1 (cold-clock region): loads were issued up-front,
                    # groups run early at locals 4-9.
                    if h == 0:
                        if local == 2:
                            emit_load_v(head_state[1], 1)
                        if local in GROUP_SLOTS_H1:
                            emit_group(head_state[1], GROUP_SLOTS_H1[local])
                    else:
                        if local == 2:
                            head_state[h + 1] = emit_load_k(h + 1)
                            head_state[h + 1]["touch"] = []
                        elif local == 3:
                            emit_load_q(head_state[h + 1], h + 1)
                        elif local == 6:
                            emit_load_v(head_state[h + 1], h + 1)
                        # groups spaced 2 slots apart in deadline order: the
                        # bufs=1 stage bank serializes transpose(g+1) behind
                        # group g's DVE copy, and 1-slot spacing made the
                        # in-order PE queue eat that wait. Q-group g's qt
                        # columns are first read at chunk g of the next head
                        # (locals 20/22/26/32); K8-15 at local 26.
                        if local in GROUP_SLOTS:
                            emit_group(head_state[h + 1],
                                       GROUP_SLOTS[local])
            for ip in range(n - SKEW, n):
                run_pv(ip)
                flush_outs(ip)
            flush_outs(10 ** 9)

    split_waits(nc)
    return nc


_CACHED = {}


def kernel(Q: np.ndarray, K: np.ndarray, V: np.ndarray) -> np.ndarray:
    res = _run(Q, K, V, trace=False)
    return res[0]


def _run(Q, K, V, trace=False):
    Qf = np.ascontiguousarray(Q.reshape(B * H, S, D), dtype=np.float32)
    Kf = np.ascontiguousarray(K.reshape(B * H, S, D), dtype=np.float32)
    Vf = np.ascontiguousarray(V.reshape(B * H, S, D), dtype=np.float32)

    in_maps = []
    for c in range(N_CORES):
        sl = slice(c * HEADS_PER_CORE, (c + 1) * HEADS_PER_CORE)
        in_maps.append({
            "Q": np.ascontiguousarray(Qf[sl]),
            "K": np.ascontiguousarray(Kf[sl]),
            "V": np.ascontiguousarray(Vf[sl]),
        })

    if "nc" not in _CACHED:
        _CACHED["nc"] = build_kernel()
    nc = _CACHED["nc"]

    res = run_bass_kernel_spmd(
        nc, in_maps, core_ids=list(range(N_CORES)), trace=trace
    )
    out = np.empty((B * H, S, D), dtype=np.float32)
    for c in range(N_CORES):
        out[c * HEADS_PER_CORE:(c + 1) * HEADS_PER_CORE] = res.results[c]["O"]
    return out.reshape(B, H, S, D), res



# revision 43
# speedup vs baseline: 1.0086x; 1.0086x over previous
"""Causal multi-head attention on 8 Trainium2 NeuronCores (Bass/Tile).

Problem: Q,K,V [B=2, h=16, S=2048, d=64] fp32; out = softmax(QK^T/8, causal) V.

Sharding: B*h = 32 heads split 4-per-core across 8 cores (head-parallel);
each core computes full causal attention for its 4 heads.

Schedule (vs. the 185us baseline): the PE program is software-pipelined
with skew 2 over a flat global (head, chunk, pair) list so the PE never
waits on softmax: ..., QK(i), PV(i-2), QK(i+1), PV(i-1), ... Keeping the PE
continuously busy also keeps it at the 2.4GHz pstate (an idle PE throttles
to 1.2GHz, which is where most of the baseline's time went).

Engine split: the softmax exp is COLUMN-SPLIT between ACT (exact exp,
q-columns [EXP_SPLIT:512) of each half) and DVE (Schraudolph fast-exp,
columns [crop:EXP_SPLIT)) — the two engines run in parallel on every
pair, which halves the exp leg of the QK->exp->PV chain and splits the
~80us/core of exp work across both engines (ACT alone was an 86us
bottleneck). A given q-column uses ONE engine for its entire softmax
row, so the fexp bias cancels in p/l; measured absmax-rel err 1.2e-2
(gate 2e-2). Diagonal-block causal masking runs on GPSIMD as a direct
affine_select on P^T (GPSIMD cannot touch PSUM, but P^T is SBUF). DVE
keeps the PSUM->SBUF copies + output normalize. GPSIMD issues the
(casting) input DMAs, staggered at pair slots 2/3/6 so the ~1.4us SWDGE
issue instructions never queue ahead of the masks; prep transposes run
at slots 8-13. SP issues Q^T row-dup + batched output stores.

NOTE: do NOT emit pe_touch instructions that wait on in-flight DMAs in
the middle of the pair stream — the in-order PE queue stalls on them
(and detached touches produced nondeterministically wrong results);
split_waits handles multi-wait instructions instead.

QK side runs bf16 (SWDGE casting loads, PE transpose-mode); the P/V side
runs bf16: exp writes bf16 P^T directly, V' = [V | 1] is DMA-cast to bf16,
so the PV matmuls take 1 cycle/row at any crop width (fp32r would pay 4x
below 256-wide). The two QK matmuls of a pair occupy row groups h0/h64
(64-contraction each) and run CONCURRENTLY in the PE array.

Known dead ends (measured): whole-pair alternation of exp between
ACT/DVE leaves the per-pair latency chain at ~1.1us and paces the whole
kernel; merging exp instructions across pairs via one persistent
PSUM/SBUF ring tensor serializes the pipeline (Tile's overlap tracking
over a single big tile is conservative: TimelineSim 169us vs 102us);
pool-based exp merge (double-wide [128,2048] st tiles, one tile per
merged exp so per-tile deps are exact) is PSUM-capacity-blocked: the
double tile needs bufs=2 = 8 banks + ot + stage > 8, and bufs=1 makes
the next QK group wait out the full ~1.4us merged ACT exp (TimelineSim
136us); EXP_SPLIT=200 overloads DVE and regresses to 123.1us (HW);
SKEW=3 (legal with st bufs=3 / pt bufs=4) regresses to 120.0us (HW);
36 seam-filler dummy matmuls before head-1's first QK (to keep HAM warm
through its ~4us row-dup wait) regress to 117.8us — the post-seam region
is exp-chain-paced, so PE warmth does not pay for the filler time;
XBAR DMA-transpose cannot produce the [d, 128t+p] Q^T layout (AP
walk-order mismatch); PV col-tiling needs 65+65 > 128 output partitions.
Best measured: 117711 ns (this file), rel err 1.214e-2, vs 133888 ns
session baseline.

Per-head layout:
  - Q,K loaded [128, 16*64] fp32->fp32r via SWDGE; V' [128, 16*65] bf16.
  - PE transpose-mode: Q -> Q^T [64, 2048] fp32r (+SP DMA row-dup to
    64:128), K -> K^T stacked pairs [128, 8*128] fp32r; PSUM->SBUF copies
    on DVE.
  - Pair (c, t): S^T [128, 1024] PSUM (two 64-contraction matmuls, min-256
    crops), ACT exp -> P^T bf16, diag mask, PV: O'^T [65, 512] += V'_j^T @
    P^T_j (row 64 = softmax denominator l).
  - Per chunk: O'^T -> SBUF bf16 (DVE), PE transpose to [128, 4*96] PSUM,
    one batched reciprocal + one broadcast multiply (DVE), one batched
    output store (SP).
"""

import numpy as np

import concourse.bass as bass
import concourse.bass_utils as _bass_utils
import concourse.mybir as mybir
import concourse.tile as tile
from concourse.bass_utils import run_bass_kernel_spmd
from concourse.tile import add_dep_helper

# NOTE: walrus's --enable-ldw-opt=true was tried to dedup/overlap the
# per-matmul LDWEIGHTS (~53us/core serial) but miscompiles this kernel
# (NaN output) — the flag stays at the default false.

N_CORES = 8
B, H, S, D = 2, 16, 2048, 64
HEADS_PER_CORE = (B * H) // N_CORES  # 4
NT = S // 128           # 16 k/q tiles per head
NCHUNK = S // 512       # 4 q-chunks per head
PAIRS_PER_HEAD = sum(2 * c + 2 for c in range(NCHUNK))  # 20
F32 = mybir.dt.float32
F32R = mybir.dt.float32r
BF16 = mybir.dt.bfloat16
I16 = mybir.dt.int16

# Schraudolph fast-exp (bf16 domain): exp(0.125*s) ~= bitcast_bf16(int16(
# s*K1 + K2)). Used only for pairs selected by FEXP_PATTERN.
FEXP_K1 = float(np.float32(0.125 * 1.4426950408889634 * 128))
FEXP_K2 = float(np.float32((127.0 - 0.04367744) * 128))
# Column-split exp: every pair's softmax exp is split by q-columns between
# ACT (exact exp, columns [SPLIT:512] of each 512-half) and DVE (Schraudolph
# fast-exp, columns [crop:SPLIT]). Per-q-column consistency: a given q uses
# one engine for ALL its k across the whole row, so the fexp bias cancels in
# p/l and the net error stays ~1e-3.
EXP_SPLIT = 176  # DVE takes [a:176) of each half; ACT takes [176:512)


class SplitDrainTileContext(tile.TileContext):
    """TileContext whose tail drain splits its semaphore waits across
    single-wait SP nops — the TPB CTRL_NO struct holds one wait slot, so
    a drain waiting on >1 proc fails walrus codegen."""

    def _drain_and_barrier(self, tick_clock, wait_clock):
        import bass_rust
        from concourse.vector_clock import ScopedClock

        gc = tick_clock.global_clock
        for i, v in enumerate(list(gc)):
            if v <= 0:
                continue
            c = bass_rust.VectorClock()
            c.require_at_least(i, v)
            nop = self.nc.sync.nop(hint="preDrain", nofuse=True)
            wait_clock.add_sem_waits(nop.ins, ScopedClock({None: c}))
        drain_inst = self.nc.sync.drain()
        wait_clock.add_sem_waits(
            drain_inst.ins, ScopedClock({None: bass_rust.VectorClock()})
        )
        self.nc.all_engine_barrier()
        assert self.sems is not None
        popped = self.nc._tile_sem_poison_stack.pop()
        assert popped is self._sem_poison
        self.nc.clear_and_free_semaphores(list(self.sems.allocated().values()))
        self.nc.all_engine_barrier()


def pe_touch(nc, ap):
    """1-column bf16 ldweights reading `ap` — engine-level PE instruction
    that absorbs a producer's sync wait into the PE engine clock so that
    following 4-byte matmuls need at most one wait (walrus S3_LW limit)."""
    return nc.tensor.ldweights(ap.bitcast(mybir.dt.bfloat16))


def split_waits(nc):
    """Post-pass: every TPB instruction holds exactly ONE sync-wait slot;
    walrus codegen rejects more. Move extra waits onto inserted same-engine
    nofuse nops placed immediately before the instruction."""
    cnt = 0
    for fn in nc.m.functions:
        for bb in fn.blocks:
            lst = bb.instructions
            i = 0
            while i < len(lst):
                ins = lst[i]
                si = ins.sync_info
                if si is not None and si.on_wait and len(si.on_wait) > 1:
                    waits = list(si.on_wait)
                    for w in waits[:-1]:
                        nop = mybir.InstNoOp(name=f"wsplit_{cnt}", ins=[], outs=[])
                        cnt += 1
                        nop.engine = ins.engine
                        nop.bass_nofuse = True
                        nop.sync_info = mybir.SyncInfo(on_wait=[w], on_update=[])
                        lst.insert(i, nop)
                        i += 1
                    si.on_wait = [waits[-1]]
                i += 1
    return cnt


def build_kernel():
    nc = bass.Bass(trn_type="TRN2")
    q_d = nc.dram_tensor("Q", [HEADS_PER_CORE, S, D], F32, kind="ExternalInput")
    k_d = nc.dram_tensor("K", [HEADS_PER_CORE, S, D], F32, kind="ExternalInput")
    v_d = nc.dram_tensor("V", [HEADS_PER_CORE, S, D], F32, kind="ExternalInput")
    o_d = nc.dram_tensor("O", [HEADS_PER_CORE, S, D], F32, kind="ExternalOutput")

    with SplitDrainTileContext(nc) as tc:
        import contextlib

        with contextlib.ExitStack() as ctx:
            consts = ctx.enter_context(tc.tile_pool(name="consts", bufs=1))
            in_pool = ctx.enter_context(tc.tile_pool(name="in", bufs=2))
            v_pool = ctx.enter_context(tc.tile_pool(name="vp", bufs=2))
            qt_pool = ctx.enter_context(tc.tile_pool(name="qt", bufs=2))
            kt_pool = ctx.enter_context(tc.tile_pool(name="kt", bufs=2))
            pt_pool = ctx.enter_context(tc.tile_pool(name="pt", bufs=4))
            otsb_pool = ctx.enter_context(tc.tile_pool(name="otsb", bufs=2))
            ob_pool = ctx.enter_context(tc.tile_pool(name="ob", bufs=2))
            r_pool = ctx.enter_context(tc.tile_pool(name="recip", bufs=4))

            st_ps = ctx.enter_context(tc.tile_pool(name="stps", bufs=3, space="PSUM"))
            ot_ps = ctx.enter_context(tc.tile_pool(name="otps", bufs=1, space="PSUM"))
            stage_ps = ctx.enter_context(tc.tile_pool(name="stage", bufs=1, space="PSUM"))

            # ---- constants ----
            ident_f = consts.tile([128, 128], F32, tag="ident_f")
            nc.gpsimd.memset(ident_f[:], 0.0)
            nc.gpsimd.affine_select(
                out=ident_f[:], in_=ident_f[:],
                compare_op=mybir.AluOpType.not_equal, fill=1.0, base=0,
                pattern=[[-1, 128]], channel_multiplier=1,
            )
            ident_r = consts.tile([128, 128], F32R, tag="ident_r")
            nc.vector.tensor_copy(ident_r[:], ident_f[:])
            ident_b = consts.tile([128, 128], BF16, tag="ident_b")
            nc.vector.tensor_copy(ident_b[:], ident_f[:])
            # 0/1 causal keep-mask for one diagonal block of P^T [k, q]:
            # keep (1.0) where q >= k i.e. f >= p, zero where f < p.
            tmask = consts.tile([128, 128], BF16, tag="tmask")
            nc.gpsimd.memset(tmask[:], 1.0)
            # keep 1.0 where f - p + 1 > 0 i.e. q >= k; fill 0.0 above diag
            nc.gpsimd.affine_select(
                out=tmask[:], in_=tmask[:],
                compare_op=mybir.AluOpType.is_gt, fill=0.0, base=1,
                pattern=[[1, 128]], channel_multiplier=-1,
            )
            t_if = pe_touch(nc, ident_f[0:1, 0:1])
            t_ir = pe_touch(nc, ident_r[0:1, 0:1])
            t_ib = pe_touch(nc, ident_b[0:1, 0:1])
            # PE warm-up: keep the array busy early so the pstate ramps to
            # full clock while the first loads land.
            warm = stage_ps.tile([128, 512], F32, tag="stage")
            for _ in range(36):
                nc.tensor.matmul(
                    warm[:, 0:256],
                    ident_f[:, 0:64].bitcast(mybir.dt.bfloat16),
                    ident_f[:, 0:128].bitcast(mybir.dt.bfloat16),
                    start=True, stop=True,
                )

            # ---- per-head prep pieces ----
            # Loads are staggered across pair slots (k, then q, then v) so
            # the ~1.4us SWDGE issue instructions on GPSIMD never queue up
            # in front of the diagonal masks, and the PE touches (which
            # carry the DMA-completion waits) are emitted only once the
            # loads have had several pair-slots to land — emitting them
            # with the loads stalls the in-order PE queue for the full DMA
            # latency at every head boundary.
            def emit_load_k(h):
                qn = in_pool.tile([128, NT * 64], BF16, tag="qn")
                kn = in_pool.tile([128, NT * 64], BF16, tag="kn")
                nc.gpsimd.dma_start(
                    kn[:].rearrange("p (t d) -> p t d", d=64),
                    k_d[h].rearrange("(t p) d -> p t d", p=128),
                )
                qt = qt_pool.tile([128, S], BF16, tag="qt")
                kt = kt_pool.tile([128, 8 * 128], BF16, tag="kt")
                return {"qn": qn, "kn": kn, "qt": qt, "kt": kt,
                        "first_tr": None}

            def emit_load_q(hs, h):
                nc.gpsimd.dma_start(
                    hs["qn"][:].rearrange("p (t d) -> p t d", d=64),
                    q_d[h].rearrange("(t p) d -> p t d", p=128),
                )

            def emit_load_v(hs, h):
                vp = v_pool.tile([128, NT * 65], BF16, tag="vp")
                vp3 = vp[:].rearrange("p (t e) -> p t e", e=65)
                nc.gpsimd.dma_start(
                    vp3[:, :, 0:64],
                    v_d[h].rearrange("(t p) d -> p t d", p=128),
                )
                nc.gpsimd.memset(vp3[:, :, 64:65], 1.0)
                hs["vp"] = vp

            def emit_loads(h):
                hs = emit_load_k(h)
                emit_load_q(hs, h)
                emit_load_v(hs, h)
                hs["touch"] = []
                return hs

            def emit_group(hs, g, dup_eng=None):
                """g 0..3: Q transpose groups, bf16 PE transpose-mode
                (DVE copy + SP row-dup); g 4..5: K^T stacked pairs, bf16
                PE transpose-mode (DVE copy). An XBAR DMA-transpose K was
                tried: each occupies the Sync engine ~1.2us and starves
                the PE at head boundaries — PE transposes are cheaper."""
                if g < 4:
                    stage = stage_ps.tile([128, 512], BF16, tag="stage",
                                          name="stage")
                    for s_i in range(4):
                        b = 4 * g + s_i
                        mm = nc.tensor.transpose(
                            stage[0:64, 128 * s_i:128 * s_i + 128],
                            hs["qn"][:, 64 * b:64 * b + 64],
                            ident_b[0:128, 0:128],
                        )
                        if hs["first_tr"] is None:
                            hs["first_tr"] = mm
                            for t in [t_if, t_ir, t_ib] + hs["touch"]:
                                if t is not None:
                                    add_dep_helper(mm.ins, t.ins, sync=False,
                                                   reason="presync")
                    nc.vector.tensor_copy(
                        hs["qt"][0:64, 512 * g:512 * g + 512],
                        stage[0:64, :],
                    )
                    # row-dup; head-1's Q0 dup is issued on ACT's HWDGE
                    # to jump the in-order SP queue (it sits behind head-0's
                    # four dups there, firing ~4us after the next head's
                    # first QK needs it and re-throttling HAM).
                    (dup_eng or nc.sync).dma_start(
                        hs["qt"][64:128, 512 * g:512 * g + 512],
                        hs["qt"][0:64, 512 * g:512 * g + 512],
                    )
                else:
                    gg = g - 4
                    stage = stage_ps.tile([128, 512], BF16, tag="stage",
                                          name="stage")
                    for s_i in range(4):
                        t_i = 4 * gg + s_i
                        mm = nc.tensor.transpose(
                            stage[:, 128 * s_i:128 * s_i + 128],
                            hs["kn"][:, 128 * t_i:128 * t_i + 128],
                            ident_b[0:128, 0:128],
                        )
                        if hs["first_tr"] is None:
                            hs["first_tr"] = mm
                            for t in [t_if, t_ir, t_ib] + hs["touch"]:
                                if t is not None:
                                    add_dep_helper(mm.ins, t.ins, sync=False,
                                                   reason="presync")
                    nc.vector.tensor_copy(
                        hs["kt"][:, 512 * gg:512 * gg + 512], stage[:, :]
                    )
                if g == 3:
                    hs["tq1"] = pe_touch(nc, hs["qt"][0:1, 0:1])
                    hs["tk1"] = pe_touch(nc, hs["kt"][0:1, 0:1])

            # ---- pair ops ----
            exp_ctr = [0]

            def emit_qk(hs, h, c, t, first_of_head):
                qt, kt = hs["qt"], hs["kt"]
                j1, j2 = 2 * t, 2 * t + 1
                cA = 128 * j1 - 512 * c
                cB = 128 * j2 - 512 * c
                a1 = max(0, cA)
                a2 = max(0, cB)
                st = st_ps.tile([128, 1024], F32, tag="st")
                mmA = nc.tensor.matmul(
                    st[:, a1:512],
                    kt[0:64, 128 * t:128 * t + 128],
                    qt[0:64, 512 * c + a1:512 * c + 512],
                    start=True, stop=True,
                )
                if first_of_head:
                    for tt in (hs.get("tq1"), hs.get("tk1")):
                        if tt is not None:
                            add_dep_helper(mmA.ins, tt.ins, sync=False,
                                           reason="presync")
                nc.tensor.matmul(
                    st[:, 512 + a2:1024],
                    kt[64:128, 128 * t:128 * t + 128],
                    qt[64:128, 512 * c + a2:512 * c + 512],
                    start=True, stop=True,
                )

                pt = pt_pool.tile([128, 1024], BF16, tag="pt")
                exp_ctr[0] += 1
                # [128, 2, 512] half-major views of P^T / S^T; half 1's
                # [a1:a2) sliver is over-computed as before (harmless).
                pv2 = pt[:].rearrange("p (h x) -> p h x", x=512)
                pi2 = pt[:].bitcast(I16).rearrange("p (h x) -> p h x", x=512)
                sv2 = st[:].rearrange("p (h x) -> p h x", x=512)
                lo = a1
                if lo < EXP_SPLIT - 32:
                    nc.vector.tensor_scalar(
                        pi2[:, :, lo:EXP_SPLIT],
                        sv2[:, :, lo:EXP_SPLIT],
                        FEXP_K1, FEXP_K2,
                        mybir.AluOpType.mult, mybir.AluOpType.add,
                    )
                    m = EXP_SPLIT
                else:
                    m = lo
                nc.scalar.activation(
                    pv2[:, :, m:512], sv2[:, :, m:512],
                    mybir.ActivationFunctionType.Exp, scale=0.125,
                )
                # zero the in-block upper triangles of diagonal tiles: the
                # last two pairs of each chunk hold them, at in-pair col
                # offsets (0, 640) for pair 2c and (256, 896) for pair 2c+1.
                npair = 2 * c + 2
                if t >= npair - 2:
                    off = 0 if t == npair - 2 else 256
                    v8 = pt[:].rearrange("p (i x) -> p i x", x=128)
                    i0 = off // 128
                    dview = v8[:, i0:i0 + 6:5, :]
                    # zero where q < k i.e. f - p + 1 <= 0; runs on GPSIMD
                    # (SBUF-only engine) to keep DVE free for fast-exp.
                    nc.gpsimd.affine_select(
                        out=dview, in_=dview,
                        compare_op=mybir.AluOpType.is_gt, fill=0.0, base=1,
                        pattern=[[0, 2], [1, 128]], channel_multiplier=-1,
                    )
                return {"st": st, "pt": pt}

            def emit_pv(hs, h, c, t, tiles, ot_holder):
                pt, vp = tiles["pt"], hs["vp"]
                npair = 2 * c + 2
                if t == 0:
                    ot_holder["ot"] = ot_ps.tile([65, 512], F32, tag="ot",
                                                 name="ot")
                ot = ot_holder["ot"]
                for half, j in enumerate((2 * t, 2 * t + 1)):
                    vA = max(0, 128 * j - 512 * c)
                    nc.tensor.matmul(
                        ot[:, vA:512],
                        vp[:, 65 * j:65 * j + 65],
                        pt[:, 512 * half + vA:512 * half + 512],
                        start=(t == 0 and half == 0),
                        stop=(t == npair - 1 and half == 1),
                        skip_group_check=True,
                    )

            def emit_out_copy(ot_holder):
                ot = ot_holder["ot"]
                otsb = otsb_pool.tile([65, 512], F32R, tag="otsb")
                nc.vector.tensor_copy(otsb[:], ot[:])
                ot_holder["otsb"] = otsb

            def emit_out(hs, h, c, ot_holder):
                otsb = ot_holder["otsb"]
                oq = stage_ps.tile([128, 384], F32R, tag="stage", name="oq")
                for i in range(4):
                    nc.tensor.transpose(
                        oq[:, 96 * i:96 * i + 96],
                        otsb[0:65, 128 * i:128 * i + 128],
                        ident_r[0:65, 0:96],
                    )
                oq4 = oq[:].bitcast(F32).rearrange("p (i x) -> p i x", x=96)
                rec = r_pool.tile([128, 4], F32, tag="rec")
                nc.vector.reciprocal(rec[:][:, :, None], oq4[:, :, 64:65])
                ob = ob_pool.tile([128, 256], F32, tag="ob")
                nc.vector.tensor_tensor(
                    ob[:].rearrange("p (i x) -> p i x", x=64),
                    oq4[:, :, 0:64],
                    rec[:].broadcast_to([128, 4, 64]),
                    mybir.AluOpType.mult,
                )
                nc.sync.dma_start(
                    o_d[h, 512 * c:512 * c + 512, :].rearrange(
                        "(t p) d -> p t d", p=128),
                    ob[:].rearrange("p (t d) -> p t d", d=64),
                )

            # ---- flat skew-2 pipeline over all (head, chunk, pair) ----
            all_pairs = []
            for h in range(HEADS_PER_CORE):
                for c in range(NCHUNK):
                    for t in range(2 * c + 2):
                        all_pairs.append((h, c, t))

            # K^T groups first so kt is ready when a head's first QK fires;
            # head 0 interleaves its last Q groups into its first pairs.
            PREP_ORDER = [4, 5, 0, 1, 2, 3]
            # need-order K0, Q0, K8, Q1, Q2, Q3 spaced 2 slots apart
            GROUP_SLOTS = {8: 4, 10: 0, 12: 5, 14: 1, 16: 2, 18: 3}
            GROUP_SLOTS_H1 = {4: 4, 6: 0, 8: 5, 10: 1, 12: 2, 14: 3}
            head_state = [None] * HEADS_PER_CORE
            head_state[0] = emit_loads(0)
            # head 1's K/Q loads issue up-front: head 0's early pairs run
            # at the cold 1.2GHz clock, so slot-based prep lead is too
            # short in wall time — without this, head 1's first QKs wait
            # ~3.6us on the row-dup DMAs and re-trigger HAM cold.
            head_state[1] = emit_load_k(1)
            head_state[1]["touch"] = []
            emit_load_q(head_state[1], 1)
            for g in PREP_ORDER[:3]:
                emit_group(head_state[0], g)

            tiles_by_idx = {}
            ot_holders = {}
            out_queue = []  # (due_slot, h, c, holder): PE out-part delayed
            n = len(all_pairs)
            SKEW = 2
            OUT_DELAY = 0

            def run_pv(ip):
                hp, cp, tp = all_pairs[ip]
                key = (hp, cp)
                if key not in ot_holders:
                    ot_holders[key] = {}
                emit_pv(head_state[hp], hp, cp, tp, tiles_by_idx.pop(ip),
                        ot_holders[key])
                if tp == 2 * cp + 1:
                    holder = ot_holders.pop(key)
                    emit_out_copy(holder)
                    out_queue.append([ip + OUT_DELAY, hp, cp, holder])

            def flush_outs(slot):
                while out_queue and out_queue[0][0] <= slot:
                    _, hp, cp, holder = out_queue.pop(0)
                    emit_out(head_state[hp], hp, cp, holder)

            for i, (h, c, t) in enumerate(all_pairs):
                local = i - PAIRS_PER_HEAD * h
                tiles_by_idx[i] = emit_qk(
                    head_state[h], h, c, t, first_of_head=(local == 0))
                if h == 0 and local in (0, 2, 4):
                    emit_group(head_state[0], PREP_ORDER[3 + local // 2])
                if i >= SKEW:
                    run_pv(i - SKEW)
                    flush_outs(i - SKEW)
                if h + 1 < HEADS_PER_CORE:
                    # loads on GPSIMD avoid mask slots (0,1,4,5,10,11,18,19);
                    # groups start once kn/qn have landed so the in-order PE
                    # queue never waits long, and the row-dup DMAs finish
                    # well before the next head's first QK needs them.
                    # Head 1 (cold-clock region): loads were issued up-front,
                    # groups run early at locals 4-9.
                    if h == 0:
                        if local == 2:
                            emit_load_v(head_state[1], 1)
                        if local in GROUP_SLOTS_H1:
                            g1 = GROUP_SLOTS_H1[local]
                            emit_group(head_state[1], g1,
                                       dup_eng=nc.scalar if g1 == 0 else None)
                    else:
                        if local == 2:
                            head_state[h + 1] = emit_load_k(h + 1)
                            head_state[h + 1]["touch"] = []
                        elif local == 3:
                            emit_load_q(head_state[h + 1], h + 1)
                        elif local == 6:
                            emit_load_v(head_state[h + 1], h + 1)
                        # groups spaced 2 slots apart in deadline order: the
                        # bufs=1 stage bank serializes transpose(g+1) behind
                        # group g's DVE copy, and 1-slot spacing made the
                        # in-order PE queue eat that wait. Q-group g's qt
                        # columns are first read at chunk g of the next head
                        # (locals 20/22/26/32); K8-15 at local 26.
                        if local in GROUP_SLOTS:
                            emit_group(head_state[h + 1],
                                       GROUP_SLOTS[local])
            for ip in range(n - SKEW, n):
                run_pv(ip)
                flush_outs(ip)
            flush_outs(10 ** 9)

    split_waits(nc)
    return nc


_CACHED = {}


def kernel(Q: np.ndarray, K: np.ndarray, V: np.ndarray) -> np.ndarray:
    res = _run(Q, K, V, trace=False)
    return res[0]


def _run(Q, K, V, trace=False):
    Qf = np.ascontiguousarray(Q.reshape(B * H, S, D), dtype=np.float32)
    Kf = np.ascontiguousarray(K.reshape(B * H, S, D), dtype=np.float32)
    Vf = np.ascontiguousarray(V.reshape(B * H, S, D), dtype=np.float32)

    in_maps = []
    for c in range(N_CORES):
        sl = slice(c * HEADS_PER_CORE, (c + 1) * HEADS_PER_CORE)
        in_maps.append({
            "Q": np.ascontiguousarray(Qf[sl]),
            "K": np.ascontiguousarray(Kf[sl]),
            "V": np.ascontiguousarray(Vf[sl]),
        })

    if "nc" not in _CACHED:
        _CACHED["nc"] = build_kernel()
    nc = _CACHED["nc"]

    res = run_bass_kernel_spmd(
        nc, in_maps, core_ids=list(range(N_CORES)), trace=trace
    )
    out = np.empty((B * H, S, D), dtype=np.float32)
    for c in range(N_CORES):
        out[c * HEADS_PER_CORE:(c + 1) * HEADS_PER_CORE] = res.results[c]["O"]
    return out.reshape(B, H, S, D), res



# revision 45
# speedup vs baseline: 1.0151x; 1.0065x over previous
"""Causal multi-head attention on 8 Trainium2 NeuronCores (Bass/Tile).

Problem: Q,K,V [B=2, h=16, S=2048, d=64] fp32; out = softmax(QK^T/8, causal) V.

Sharding: B*h = 32 heads split 4-per-core across 8 cores (head-parallel);
each core computes full causal attention for its 4 heads.

Schedule (vs. the 185us baseline): the PE program is software-pipelined
with skew 2 over a flat global (head, chunk, pair) list so the PE never
waits on softmax: ..., QK(i), PV(i-2), QK(i+1), PV(i-1), ... Keeping the PE
continuously busy also keeps it at the 2.4GHz pstate (an idle PE throttles
to 1.2GHz, which is where most of the baseline's time went).

Engine split: the softmax exp is COLUMN-SPLIT between ACT (exact exp,
q-columns [EXP_SPLIT:512) of each half) and DVE (Schraudolph fast-exp,
columns [crop:EXP_SPLIT)) — the two engines run in parallel on every
pair, which halves the exp leg of the QK->exp->PV chain and splits the
~80us/core of exp work across both engines (ACT alone was an 86us
bottleneck). A given q-column uses ONE engine for its entire softmax
row, so the fexp bias cancels in p/l; measured absmax-rel err 1.2e-2
(gate 2e-2). Diagonal-block causal masking runs on GPSIMD as a direct
affine_select on P^T (GPSIMD cannot touch PSUM, but P^T is SBUF). DVE
keeps the PSUM->SBUF copies + output normalize. GPSIMD issues the
(casting) input DMAs, staggered at pair slots 2/3/6 so the ~1.4us SWDGE
issue instructions never queue ahead of the masks; prep transposes run
at slots 8-13. SP issues Q^T row-dup + batched output stores.

NOTE: do NOT emit pe_touch instructions that wait on in-flight DMAs in
the middle of the pair stream — the in-order PE queue stalls on them
(and detached touches produced nondeterministically wrong results);
split_waits handles multi-wait instructions instead.

QK side runs bf16 (SWDGE casting loads, PE transpose-mode); the P/V side
runs bf16: exp writes bf16 P^T directly, V' = [V | 1] is DMA-cast to bf16,
so the PV matmuls take 1 cycle/row at any crop width (fp32r would pay 4x
below 256-wide). The two QK matmuls of a pair occupy row groups h0/h64
(64-contraction each) and run CONCURRENTLY in the PE array.

Known dead ends (measured): whole-pair alternation of exp between
ACT/DVE leaves the per-pair latency chain at ~1.1us and paces the whole
kernel; merging exp instructions across pairs via one persistent
PSUM/SBUF ring tensor serializes the pipeline (Tile's overlap tracking
over a single big tile is conservative: TimelineSim 169us vs 102us);
pool-based exp merge (double-wide [128,2048] st tiles, one tile per
merged exp so per-tile deps are exact) is PSUM-capacity-blocked: the
double tile needs bufs=2 = 8 banks + ot + stage > 8, and bufs=1 makes
the next QK group wait out the full ~1.4us merged ACT exp (TimelineSim
136us); EXP_SPLIT=200 overloads DVE and regresses to 123.1us (HW);
SKEW=3 (legal with st bufs=3 / pt bufs=4) regresses to 120.0us (HW);
36 seam-filler dummy matmuls before head-1's first QK (to keep HAM warm
through its ~4us row-dup wait) regress to 117.8us — the post-seam region
is exp-chain-paced, so PE warmth does not pay for the filler time;
issuing head-1's Q0 row-dup on ACT's HWDGE to jump the in-order SP
queue regresses to 119.4us (HW) — the 632ns issue displaces early exps
at cold pace and cascades worse than the dup wait it removes;
XBAR DMA-transpose cannot produce the [d, 128t+p] Q^T layout (AP
walk-order mismatch); PV col-tiling needs 65+65 > 128 output partitions.
Best measured: 117711 ns (this file), rel err 1.214e-2, vs 133888 ns
session baseline.

Per-head layout:
  - Q,K loaded [128, 16*64] fp32->fp32r via SWDGE; V' [128, 16*65] bf16.
  - PE transpose-mode: Q -> Q^T [64, 2048] fp32r (+SP DMA row-dup to
    64:128), K -> K^T stacked pairs [128, 8*128] fp32r; PSUM->SBUF copies
    on DVE.
  - Pair (c, t): S^T [128, 1024] PSUM (two 64-contraction matmuls, min-256
    crops), ACT exp -> P^T bf16, diag mask, PV: O'^T [65, 512] += V'_j^T @
    P^T_j (row 64 = softmax denominator l).
  - Per chunk: O'^T -> SBUF bf16 (DVE), PE transpose to [128, 4*96] PSUM,
    one batched reciprocal + one broadcast multiply (DVE), one batched
    output store (SP).
"""

import numpy as np

import concourse.bass as bass
import concourse.bass_utils as _bass_utils
import concourse.mybir as mybir
import concourse.tile as tile
from concourse.bass_utils import run_bass_kernel_spmd
from concourse.tile import add_dep_helper

# NOTE: walrus's --enable-ldw-opt=true was tried to dedup/overlap the
# per-matmul LDWEIGHTS (~53us/core serial) but miscompiles this kernel
# (NaN output) — the flag stays at the default false.

N_CORES = 8
B, H, S, D = 2, 16, 2048, 64
HEADS_PER_CORE = (B * H) // N_CORES  # 4
NT = S // 128           # 16 k/q tiles per head
NCHUNK = S // 512       # 4 q-chunks per head
PAIRS_PER_HEAD = sum(2 * c + 2 for c in range(NCHUNK))  # 20
F32 = mybir.dt.float32
F32R = mybir.dt.float32r
BF16 = mybir.dt.bfloat16
I16 = mybir.dt.int16

# Schraudolph fast-exp (bf16 domain): exp(0.125*s) ~= bitcast_bf16(int16(
# s*K1 + K2)). Used only for pairs selected by FEXP_PATTERN.
FEXP_K1 = float(np.float32(0.125 * 1.4426950408889634 * 128))
FEXP_K2 = float(np.float32((127.0 - 0.04367744) * 128))
# Column-split exp: every pair's softmax exp is split by q-columns between
# ACT (exact exp, columns [SPLIT:512] of each 512-half) and DVE (Schraudolph
# fast-exp, columns [crop:SPLIT]). Per-q-column consistency: a given q uses
# one engine for ALL its k across the whole row, so the fexp bias cancels in
# p/l and the net error stays ~1e-3.
EXP_SPLIT = 176  # DVE takes [a:176) of each half; ACT takes [176:512)


class SplitDrainTileContext(tile.TileContext):
    """TileContext whose tail drain splits its semaphore waits across
    single-wait SP nops — the TPB CTRL_NO struct holds one wait slot, so
    a drain waiting on >1 proc fails walrus codegen."""

    def _drain_and_barrier(self, tick_clock, wait_clock):
        import bass_rust
        from concourse.vector_clock import ScopedClock

        gc = tick_clock.global_clock
        for i, v in enumerate(list(gc)):
            if v <= 0:
                continue
            c = bass_rust.VectorClock()
            c.require_at_least(i, v)
            nop = self.nc.sync.nop(hint="preDrain", nofuse=True)
            wait_clock.add_sem_waits(nop.ins, ScopedClock({None: c}))
        drain_inst = self.nc.sync.drain()
        wait_clock.add_sem_waits(
            drain_inst.ins, ScopedClock({None: bass_rust.VectorClock()})
        )
        self.nc.all_engine_barrier()
        assert self.sems is not None
        popped = self.nc._tile_sem_poison_stack.pop()
        assert popped is self._sem_poison
        self.nc.clear_and_free_semaphores(list(self.sems.allocated().values()))
        self.nc.all_engine_barrier()


def pe_touch(nc, ap):
    """1-column bf16 ldweights reading `ap` — engine-level PE instruction
    that absorbs a producer's sync wait into the PE engine clock so that
    following 4-byte matmuls need at most one wait (walrus S3_LW limit)."""
    return nc.tensor.ldweights(ap.bitcast(mybir.dt.bfloat16))


def split_waits(nc):
    """Post-pass: every TPB instruction holds exactly ONE sync-wait slot;
    walrus codegen rejects more. Move extra waits onto inserted same-engine
    nofuse nops placed immediately before the instruction."""
    cnt = 0
    for fn in nc.m.functions:
        for bb in fn.blocks:
            lst = bb.instructions
            i = 0
            while i < len(lst):
                ins = lst[i]
                si = ins.sync_info
                if si is not None and si.on_wait and len(si.on_wait) > 1:
                    waits = list(si.on_wait)
                    for w in waits[:-1]:
                        nop = mybir.InstNoOp(name=f"wsplit_{cnt}", ins=[], outs=[])
                        cnt += 1
                        nop.engine = ins.engine
                        nop.bass_nofuse = True
                        nop.sync_info = mybir.SyncInfo(on_wait=[w], on_update=[])
                        lst.insert(i, nop)
                        i += 1
                    si.on_wait = [waits[-1]]
                i += 1
    return cnt


def build_kernel():
    nc = bass.Bass(trn_type="TRN2")
    q_d = nc.dram_tensor("Q", [HEADS_PER_CORE, S, D], F32, kind="ExternalInput")
    k_d = nc.dram_tensor("K", [HEADS_PER_CORE, S, D], F32, kind="ExternalInput")
    v_d = nc.dram_tensor("V", [HEADS_PER_CORE, S, D], F32, kind="ExternalInput")
    o_d = nc.dram_tensor("O", [HEADS_PER_CORE, S, D], F32, kind="ExternalOutput")

    with SplitDrainTileContext(nc) as tc:
        import contextlib

        with contextlib.ExitStack() as ctx:
            consts = ctx.enter_context(tc.tile_pool(name="consts", bufs=1))
            in_pool = ctx.enter_context(tc.tile_pool(name="in", bufs=2))
            v_pool = ctx.enter_context(tc.tile_pool(name="vp", bufs=2))
            qt_pool = ctx.enter_context(tc.tile_pool(name="qt", bufs=2))
            kt_pool = ctx.enter_context(tc.tile_pool(name="kt", bufs=2))
            ktx_pool = ctx.enter_context(tc.tile_pool(name="ktx", bufs=2))
            pt_pool = ctx.enter_context(tc.tile_pool(name="pt", bufs=4))
            otsb_pool = ctx.enter_context(tc.tile_pool(name="otsb", bufs=2))
            ob_pool = ctx.enter_context(tc.tile_pool(name="ob", bufs=2))
            r_pool = ctx.enter_context(tc.tile_pool(name="recip", bufs=4))

            st_ps = ctx.enter_context(tc.tile_pool(name="stps", bufs=3, space="PSUM"))
            ot_ps = ctx.enter_context(tc.tile_pool(name="otps", bufs=1, space="PSUM"))
            stage_ps = ctx.enter_context(tc.tile_pool(name="stage", bufs=1, space="PSUM"))

            # ---- constants ----
            ident_f = consts.tile([128, 128], F32, tag="ident_f")
            nc.gpsimd.memset(ident_f[:], 0.0)
            nc.gpsimd.affine_select(
                out=ident_f[:], in_=ident_f[:],
                compare_op=mybir.AluOpType.not_equal, fill=1.0, base=0,
                pattern=[[-1, 128]], channel_multiplier=1,
            )
            ident_r = consts.tile([128, 128], F32R, tag="ident_r")
            nc.vector.tensor_copy(ident_r[:], ident_f[:])
            ident_b = consts.tile([128, 128], BF16, tag="ident_b")
            nc.vector.tensor_copy(ident_b[:], ident_f[:])
            # 0/1 causal keep-mask for one diagonal block of P^T [k, q]:
            # keep (1.0) where q >= k i.e. f >= p, zero where f < p.
            tmask = consts.tile([128, 128], BF16, tag="tmask")
            nc.gpsimd.memset(tmask[:], 1.0)
            # keep 1.0 where f - p + 1 > 0 i.e. q >= k; fill 0.0 above diag
            nc.gpsimd.affine_select(
                out=tmask[:], in_=tmask[:],
                compare_op=mybir.AluOpType.is_gt, fill=0.0, base=1,
                pattern=[[1, 128]], channel_multiplier=-1,
            )
            t_if = pe_touch(nc, ident_f[0:1, 0:1])
            t_ir = pe_touch(nc, ident_r[0:1, 0:1])
            t_ib = pe_touch(nc, ident_b[0:1, 0:1])
            # PE warm-up: keep the array busy early so the pstate ramps to
            # full clock while the first loads land.
            warm = stage_ps.tile([128, 512], F32, tag="stage")
            for _ in range(36):
                nc.tensor.matmul(
                    warm[:, 0:256],
                    ident_f[:, 0:64].bitcast(mybir.dt.bfloat16),
                    ident_f[:, 0:128].bitcast(mybir.dt.bfloat16),
                    start=True, stop=True,
                )

            # ---- per-head prep pieces ----
            # Loads are staggered across pair slots (k, then q, then v) so
            # the ~1.4us SWDGE issue instructions on GPSIMD never queue up
            # in front of the diagonal masks, and the PE touches (which
            # carry the DMA-completion waits) are emitted only once the
            # loads have had several pair-slots to land — emitting them
            # with the loads stalls the in-order PE queue for the full DMA
            # latency at every head boundary.
            def emit_load_k(h):
                qn = in_pool.tile([128, NT * 64], BF16, tag="qn")
                kn = in_pool.tile([128, NT * 64], BF16, tag="kn")
                nc.gpsimd.dma_start(
                    kn[:].rearrange("p (t d) -> p t d", d=64),
                    k_d[h].rearrange("(t p) d -> p t d", p=128),
                )
                qt = qt_pool.tile([128, S], BF16, tag="qt")
                kt = kt_pool.tile([128, 8 * 128], BF16, tag="kt")
                ktx = ktx_pool.tile([64, 256], BF16, tag="ktx")
                return {"qn": qn, "kn": kn, "qt": qt, "kt": kt, "ktx": ktx,
                        "first_tr": None}

            def emit_load_q(hs, h):
                nc.gpsimd.dma_start(
                    hs["qn"][:].rearrange("p (t d) -> p t d", d=64),
                    q_d[h].rearrange("(t p) d -> p t d", p=128),
                )

            def emit_load_v(hs, h):
                vp = v_pool.tile([128, NT * 65], BF16, tag="vp")
                vp3 = vp[:].rearrange("p (t e) -> p t e", e=65)
                nc.gpsimd.dma_start(
                    vp3[:, :, 0:64],
                    v_d[h].rearrange("(t p) d -> p t d", p=128),
                )
                nc.gpsimd.memset(vp3[:, :, 64:65], 1.0)
                hs["vp"] = vp

            def emit_loads(h):
                hs = emit_load_k(h)
                emit_load_q(hs, h)
                emit_load_v(hs, h)
                hs["touch"] = []
                return hs

            def emit_group(hs, g):
                """g 0..3: Q transpose groups, bf16 PE transpose-mode
                (DVE copy + SP row-dup); g 4..5: K^T stacked pairs, bf16
                PE transpose-mode (DVE copy). An XBAR DMA-transpose K was
                tried: each occupies the Sync engine ~1.2us and starves
                the PE at head boundaries — PE transposes are cheaper."""
                if g < 4:
                    stage = stage_ps.tile([128, 512], BF16, tag="stage",
                                          name="stage")
                    for s_i in range(4):
                        b = 4 * g + s_i
                        mm = nc.tensor.transpose(
                            stage[0:64, 128 * s_i:128 * s_i + 128],
                            hs["qn"][:, 64 * b:64 * b + 64],
                            ident_b[0:128, 0:128],
                        )
                        if hs["first_tr"] is None:
                            hs["first_tr"] = mm
                            for t in [t_if, t_ir, t_ib] + hs["touch"]:
                                if t is not None:
                                    add_dep_helper(mm.ins, t.ins, sync=False,
                                                   reason="presync")
                    nc.vector.tensor_copy(
                        hs["qt"][0:64, 512 * g:512 * g + 512],
                        stage[0:64, :],
                    )
                    # group 0's dup region qt[64:128, 0:512] is only read
                    # by chunk-0 mmBs, which now use ktx + qt[0:64] instead
                    # — skipping this dup removes the DMA that the next
                    # head's first QK stalled ~4.4us on (in-order SP queue).
                    if g != 0:
                        nc.sync.dma_start(
                            hs["qt"][64:128, 512 * g:512 * g + 512],
                            hs["qt"][0:64, 512 * g:512 * g + 512],
                        )
                else:
                    gg = g - 4
                    stage = stage_ps.tile([128, 512], BF16, tag="stage",
                                          name="stage")
                    for s_i in range(4):
                        t_i = 4 * gg + s_i
                        mm = nc.tensor.transpose(
                            stage[:, 128 * s_i:128 * s_i + 128],
                            hs["kn"][:, 128 * t_i:128 * t_i + 128],
                            ident_b[0:128, 0:128],
                        )
                        if hs["first_tr"] is None:
                            hs["first_tr"] = mm
                            for t in [t_if, t_ir, t_ib] + hs["touch"]:
                                if t is not None:
                                    add_dep_helper(mm.ins, t.ins, sync=False,
                                                   reason="presync")
                    nc.vector.tensor_copy(
                        hs["kt"][:, 512 * gg:512 * gg + 512], stage[:, :]
                    )
                    if gg == 0:
                        # odd tiles 1,3 K^T copied down to partitions 0:64
                        # (cross-partition -> DMA); issued at the K-group's
                        # early slot, ahead of all Q dups in the SP queue.
                        nc.sync.dma_start(
                            hs["ktx"][:],
                            hs["kt"][64:128, 0:256],
                        )
                if g == 3:
                    hs["tq1"] = pe_touch(nc, hs["qt"][0:1, 0:1])
                    hs["tk1"] = pe_touch(nc, hs["kt"][0:1, 0:1])

            # ---- pair ops ----
            exp_ctr = [0]

            def emit_qk(hs, h, c, t, first_of_head):
                qt, kt = hs["qt"], hs["kt"]
                j1, j2 = 2 * t, 2 * t + 1
                cA = 128 * j1 - 512 * c
                cB = 128 * j2 - 512 * c
                a1 = max(0, cA)
                a2 = max(0, cB)
                st = st_ps.tile([128, 1024], F32, tag="st")
                mmA = nc.tensor.matmul(
                    st[:, a1:512],
                    kt[0:64, 128 * t:128 * t + 128],
                    qt[0:64, 512 * c + a1:512 * c + 512],
                    start=True, stop=True,
                )
                if first_of_head:
                    for tt in (hs.get("tq1"), hs.get("tk1")):
                        if tt is not None:
                            add_dep_helper(mmA.ins, tt.ins, sync=False,
                                           reason="presync")
                if c == 0:
                    # weights from ktx (base partition 0) + stream from
                    # qt[0:64]: no dependence on the (removed) group-0 dup;
                    # serializes with mmA (same row group) — ~0.2us/head.
                    nc.tensor.matmul(
                        st[:, 512 + a2:1024],
                        hs["ktx"][:, 128 * t:128 * t + 128],
                        qt[0:64, a2:512],
                        start=True, stop=True,
                    )
                else:
                    nc.tensor.matmul(
                        st[:, 512 + a2:1024],
                        kt[64:128, 128 * t:128 * t + 128],
                        qt[64:128, 512 * c + a2:512 * c + 512],
                        start=True, stop=True,
                    )

                pt = pt_pool.tile([128, 1024], BF16, tag="pt")
                exp_ctr[0] += 1
                # [128, 2, 512] half-major views of P^T / S^T; half 1's
                # [a1:a2) sliver is over-computed as before (harmless).
                pv2 = pt[:].rearrange("p (h x) -> p h x", x=512)
                pi2 = pt[:].bitcast(I16).rearrange("p (h x) -> p h x", x=512)
                sv2 = st[:].rearrange("p (h x) -> p h x", x=512)
                lo = a1
                if lo < EXP_SPLIT - 32:
                    nc.vector.tensor_scalar(
                        pi2[:, :, lo:EXP_SPLIT],
                        sv2[:, :, lo:EXP_SPLIT],
                        FEXP_K1, FEXP_K2,
                        mybir.AluOpType.mult, mybir.AluOpType.add,
                    )
                    m = EXP_SPLIT
                else:
                    m = lo
                nc.scalar.activation(
                    pv2[:, :, m:512], sv2[:, :, m:512],
                    mybir.ActivationFunctionType.Exp, scale=0.125,
                )
                # zero the in-block upper triangles of diagonal tiles: the
                # last two pairs of each chunk hold them, at in-pair col
                # offsets (0, 640) for pair 2c and (256, 896) for pair 2c+1.
                npair = 2 * c + 2
                if t >= npair - 2:
                    off = 0 if t == npair - 2 else 256
                    v8 = pt[:].rearrange("p (i x) -> p i x", x=128)
                    i0 = off // 128
                    dview = v8[:, i0:i0 + 6:5, :]
                    # zero where q < k i.e. f - p + 1 <= 0; runs on GPSIMD
                    # (SBUF-only engine) to keep DVE free for fast-exp.
                    nc.gpsimd.affine_select(
                        out=dview, in_=dview,
                        compare_op=mybir.AluOpType.is_gt, fill=0.0, base=1,
                        pattern=[[0, 2], [1, 128]], channel_multiplier=-1,
                    )
                return {"st": st, "pt": pt}

            def emit_pv(hs, h, c, t, tiles, ot_holder):
                pt, vp = tiles["pt"], hs["vp"]
                npair = 2 * c + 2
                if t == 0:
                    ot_holder["ot"] = ot_ps.tile([65, 512], F32, tag="ot",
                                                 name="ot")
                ot = ot_holder["ot"]
                for half, j in enumerate((2 * t, 2 * t + 1)):
                    vA = max(0, 128 * j - 512 * c)
                    nc.tensor.matmul(
                        ot[:, vA:512],
                        vp[:, 65 * j:65 * j + 65],
                        pt[:, 512 * half + vA:512 * half + 512],
                        start=(t == 0 and half == 0),
                        stop=(t == npair - 1 and half == 1),
                        skip_group_check=True,
                    )

            def emit_out_copy(ot_holder):
                ot = ot_holder["ot"]
                otsb = otsb_pool.tile([65, 512], F32R, tag="otsb")
                nc.vector.tensor_copy(otsb[:], ot[:])
                ot_holder["otsb"] = otsb

            def emit_out(hs, h, c, ot_holder):
                otsb = ot_holder["otsb"]
                oq = stage_ps.tile([128, 384], F32R, tag="stage", name="oq")
                for i in range(4):
                    nc.tensor.transpose(
                        oq[:, 96 * i:96 * i + 96],
                        otsb[0:65, 128 * i:128 * i + 128],
                        ident_r[0:65, 0:96],
                    )
                oq4 = oq[:].bitcast(F32).rearrange("p (i x) -> p i x", x=96)
                rec = r_pool.tile([128, 4], F32, tag="rec")
                nc.vector.reciprocal(rec[:][:, :, None], oq4[:, :, 64:65])
                ob = ob_pool.tile([128, 256], F32, tag="ob")
                nc.vector.tensor_tensor(
                    ob[:].rearrange("p (i x) -> p i x", x=64),
                    oq4[:, :, 0:64],
                    rec[:].broadcast_to([128, 4, 64]),
                    mybir.AluOpType.mult,
                )
                nc.sync.dma_start(
                    o_d[h, 512 * c:512 * c + 512, :].rearrange(
                        "(t p) d -> p t d", p=128),
                    ob[:].rearrange("p (t d) -> p t d", d=64),
                )

            # ---- flat skew-2 pipeline over all (head, chunk, pair) ----
            all_pairs = []
            for h in range(HEADS_PER_CORE):
                for c in range(NCHUNK):
                    for t in range(2 * c + 2):
                        all_pairs.append((h, c, t))

            # K^T groups first so kt is ready when a head's first QK fires;
            # head 0 interleaves its last Q groups into its first pairs.
            PREP_ORDER = [4, 5, 0, 1, 2, 3]
            # need-order K0, Q0, K8, Q1, Q2, Q3 spaced 2 slots apart
            GROUP_SLOTS = {8: 4, 10: 0, 12: 5, 14: 1, 16: 2, 18: 3}
            GROUP_SLOTS_H1 = {4: 4, 6: 0, 8: 5, 10: 1, 12: 2, 14: 3}
            head_state = [None] * HEADS_PER_CORE
            head_state[0] = emit_loads(0)
            # head 1's K/Q loads issue up-front: head 0's early pairs run
            # at the cold 1.2GHz clock, so slot-based prep lead is too
            # short in wall time — without this, head 1's first QKs wait
            # ~3.6us on the row-dup DMAs and re-trigger HAM cold.
            head_state[1] = emit_load_k(1)
            head_state[1]["touch"] = []
            emit_load_q(head_state[1], 1)
            for g in PREP_ORDER[:3]:
                emit_group(head_state[0], g)

            tiles_by_idx = {}
            ot_holders = {}
            out_queue = []  # (due_slot, h, c, holder): PE out-part delayed
            n = len(all_pairs)
            SKEW = 2
            OUT_DELAY = 0

            def run_pv(ip):
                hp, cp, tp = all_pairs[ip]
                key = (hp, cp)
                if key not in ot_holders:
                    ot_holders[key] = {}
                emit_pv(head_state[hp], hp, cp, tp, tiles_by_idx.pop(ip),
                        ot_holders[key])
                if tp == 2 * cp + 1:
                    holder = ot_holders.pop(key)
                    emit_out_copy(holder)
                    out_queue.append([ip + OUT_DELAY, hp, cp, holder])

            def flush_outs(slot):
                while out_queue and out_queue[0][0] <= slot:
                    _, hp, cp, holder = out_queue.pop(0)
                    emit_out(head_state[hp], hp, cp, holder)

            for i, (h, c, t) in enumerate(all_pairs):
                local = i - PAIRS_PER_HEAD * h
                tiles_by_idx[i] = emit_qk(
                    head_state[h], h, c, t, first_of_head=(local == 0))
                if h == 0 and local in (0, 2, 4):
                    emit_group(head_state[0], PREP_ORDER[3 + local // 2])
                if i >= SKEW:
                    run_pv(i - SKEW)
                    flush_outs(i - SKEW)
                if h + 1 < HEADS_PER_CORE:
                    # loads on GPSIMD avoid mask slots (0,1,4,5,10,11,18,19);
                    # groups start once kn/qn have landed so the in-order PE
                    # queue never waits long, and the row-dup DMAs finish
                    # well before the next head's first QK needs them.
                    # Head 1 (cold-clock region): loads were issued up-front,
                    # groups run early at locals 4-9.
                    if h == 0:
                        if local == 2:
                            emit_load_v(head_state[1], 1)
                        if local in GROUP_SLOTS_H1:
                            emit_group(head_state[1], GROUP_SLOTS_H1[local])
                    else:
                        if local == 2:
                            head_state[h + 1] = emit_load_k(h + 1)
                            head_state[h + 1]["touch"] = []
                        elif local == 3:
                            emit_load_q(head_state[h + 1], h + 1)
                        elif local == 6:
                            emit_load_v(head_state[h + 1], h + 1)
                        # groups spaced 2 slots apart in deadline order: the
                        # bufs=1 stage bank serializes transpose(g+1) behind
                        # group g's DVE copy, and 1-slot spacing made the
                        # in-order PE queue eat that wait. Q-group g's qt
                        # columns are first read at chunk g of the next head
                        # (locals 20/22/26/32); K8-15 at local 26.
                        if local in GROUP_SLOTS:
                            emit_group(head_state[h + 1],
                                       GROUP_SLOTS[local])
            for ip in range(n - SKEW, n):
                run_pv(ip)
                flush_outs(ip)
            flush_outs(10 ** 9)

    split_waits(nc)
    return nc


_CACHED = {}


def kernel(Q: np.ndarray, K: np.ndarray, V: np.ndarray) -> np.ndarray:
    res = _run(Q, K, V, trace=False)
    return res[0]


def _run(Q, K, V, trace=False):
    Qf = np.ascontiguousarray(Q.reshape(B * H, S, D), dtype=np.float32)
    Kf = np.ascontiguousarray(K.reshape(B * H, S, D), dtype=np.float32)
    Vf = np.ascontiguousarray(V.reshape(B * H, S, D), dtype=np.float32)

    in_maps = []
    for c in range(N_CORES):
        sl = slice(c * HEADS_PER_CORE, (c + 1) * HEADS_PER_CORE)
        in_maps.append({
            "Q": np.ascontiguousarray(Qf[sl]),
            "K": np.ascontiguousarray(Kf[sl]),
            "V": np.ascontiguousarray(Vf[sl]),
        })

    if "nc" not in _CACHED:
        _CACHED["nc"] = build_kernel()
    nc = _CACHED["nc"]

    res = run_bass_kernel_spmd(
        nc, in_maps, core_ids=list(range(N_CORES)), trace=trace
    )
    out = np.empty((B * H, S, D), dtype=np.float32)
    for c in range(N_CORES):
        out[c * HEADS_PER_CORE:(c + 1) * HEADS_PER_CORE] = res.results[c]["O"]
    return out.reshape(B, H, S, D), res



# revision 47
# speedup vs baseline: 1.0290x; 1.0137x over previous
"""Causal multi-head attention on 8 Trainium2 NeuronCores (Bass/Tile).

Problem: Q,K,V [B=2, h=16, S=2048, d=64] fp32; out = softmax(QK^T/8, causal) V.

Sharding: B*h = 32 heads split 4-per-core across 8 cores (head-parallel);
each core computes full causal attention for its 4 heads.

Schedule (vs. the 185us baseline): the PE program is software-pipelined
with skew 2 over a flat global (head, chunk, pair) list so the PE never
waits on softmax: ..., QK(i), PV(i-2), QK(i+1), PV(i-1), ... Keeping the PE
continuously busy also keeps it at the 2.4GHz pstate (an idle PE throttles
to 1.2GHz, which is where most of the baseline's time went).

Engine split: the softmax exp is COLUMN-SPLIT between ACT (exact exp,
q-columns [EXP_SPLIT:512) of each half) and DVE (Schraudolph fast-exp,
columns [crop:EXP_SPLIT)) — the two engines run in parallel on every
pair, which halves the exp leg of the QK->exp->PV chain and splits the
~80us/core of exp work across both engines (ACT alone was an 86us
bottleneck). A given q-column uses ONE engine for its entire softmax
row, so the fexp bias cancels in p/l; measured absmax-rel err 1.2e-2
(gate 2e-2). Diagonal-block causal masking runs on GPSIMD as a direct
affine_select on P^T (GPSIMD cannot touch PSUM, but P^T is SBUF). DVE
keeps the PSUM->SBUF copies + output normalize. GPSIMD issues the
(casting) input DMAs, staggered at pair slots 2/3/6 so the ~1.4us SWDGE
issue instructions never queue ahead of the masks; prep transposes run
at slots 8-13. SP issues Q^T row-dup + batched output stores.

NOTE: do NOT emit pe_touch instructions that wait on in-flight DMAs in
the middle of the pair stream — the in-order PE queue stalls on them
(and detached touches produced nondeterministically wrong results);
split_waits handles multi-wait instructions instead.

QK side runs bf16 (SWDGE casting loads, PE transpose-mode); the P/V side
runs bf16: exp writes bf16 P^T directly, V' = [V | 1] is DMA-cast to bf16,
so the PV matmuls take 1 cycle/row at any crop width (fp32r would pay 4x
below 256-wide). The two QK matmuls of a pair occupy row groups h0/h64
(64-contraction each) and run CONCURRENTLY in the PE array.

Known dead ends (measured): whole-pair alternation of exp between
ACT/DVE leaves the per-pair latency chain at ~1.1us and paces the whole
kernel; merging exp instructions across pairs via one persistent
PSUM/SBUF ring tensor serializes the pipeline (Tile's overlap tracking
over a single big tile is conservative: TimelineSim 169us vs 102us);
pool-based exp merge (double-wide [128,2048] st tiles, one tile per
merged exp so per-tile deps are exact) is PSUM-capacity-blocked: the
double tile needs bufs=2 = 8 banks + ot + stage > 8, and bufs=1 makes
the next QK group wait out the full ~1.4us merged ACT exp (TimelineSim
136us); EXP_SPLIT=200 overloads DVE and regresses to 123.1us (HW);
SKEW=3 (legal with st bufs=3 / pt bufs=4) regresses to 120.0us (HW);
36 seam-filler dummy matmuls before head-1's first QK (to keep HAM warm
through its ~4us row-dup wait) regress to 117.8us — the post-seam region
is exp-chain-paced, so PE warmth does not pay for the filler time;
issuing head-1's Q0 row-dup on ACT's HWDGE to jump the in-order SP
queue regresses to 119.4us (HW) — the 632ns issue displaces early exps
at cold pace and cascades worse than the dup wait it removes;
eliminating the group-0 row-dup entirely (chunk-0 mmB weights from a
ktx[64,256] copy of the odd tiles at partitions 0:64, streaming
qt[0:64]) is numerically correct but regresses to 118.6us (HW) — the
chunk-0 QK serialization + extra DMA outweigh the removed seam wait;
XBAR DMA-transpose cannot produce the [d, 128t+p] Q^T layout (AP
walk-order mismatch); PV col-tiling needs 65+65 > 128 output partitions.
Best measured: 117711 ns (this file), rel err 1.214e-2, vs 133888 ns
session baseline.

Per-head layout:
  - Q,K loaded [128, 16*64] fp32->fp32r via SWDGE; V' [128, 16*65] bf16.
  - PE transpose-mode: Q -> Q^T [64, 2048] fp32r (+SP DMA row-dup to
    64:128), K -> K^T stacked pairs [128, 8*128] fp32r; PSUM->SBUF copies
    on DVE.
  - Pair (c, t): S^T [128, 1024] PSUM (two 64-contraction matmuls, min-256
    crops), ACT exp -> P^T bf16, diag mask, PV: O'^T [65, 512] += V'_j^T @
    P^T_j (row 64 = softmax denominator l).
  - Per chunk: O'^T -> SBUF bf16 (DVE), PE transpose to [128, 4*96] PSUM,
    one batched reciprocal + one broadcast multiply (DVE), one batched
    output store (SP).
"""

import numpy as np

import concourse.bass as bass
import concourse.bass_utils as _bass_utils
import concourse.mybir as mybir
import concourse.tile as tile
from concourse.bass_utils import run_bass_kernel_spmd
from concourse.tile import add_dep_helper

# NOTE: walrus's --enable-ldw-opt=true was tried to dedup/overlap the
# per-matmul LDWEIGHTS (~53us/core serial) but miscompiles this kernel
# (NaN output) — the flag stays at the default false.

N_CORES = 8
B, H, S, D = 2, 16, 2048, 64
HEADS_PER_CORE = (B * H) // N_CORES  # 4
NT = S // 128           # 16 k/q tiles per head
NCHUNK = S // 512       # 4 q-chunks per head
PAIRS_PER_HEAD = sum(2 * c + 2 for c in range(NCHUNK))  # 20
F32 = mybir.dt.float32
F32R = mybir.dt.float32r
BF16 = mybir.dt.bfloat16
I16 = mybir.dt.int16

# Schraudolph fast-exp (bf16 domain): exp(0.125*s) ~= bitcast_bf16(int16(
# s*K1 + K2)). Used only for pairs selected by FEXP_PATTERN.
FEXP_K1 = float(np.float32(0.125 * 1.4426950408889634 * 128))
FEXP_K2 = float(np.float32((127.0 - 0.04367744) * 128))
# Column-split exp: every pair's softmax exp is split by q-columns between
# ACT (exact exp, columns [SPLIT:512] of each 512-half) and DVE (Schraudolph
# fast-exp, columns [crop:SPLIT]). Per-q-column consistency: a given q uses
# one engine for ALL its k across the whole row, so the fexp bias cancels in
# p/l and the net error stays ~1e-3.
EXP_SPLIT = 176  # DVE takes [a:176) of each half; ACT takes [176:512)


class SplitDrainTileContext(tile.TileContext):
    """TileContext whose tail drain splits its semaphore waits across
    single-wait SP nops — the TPB CTRL_NO struct holds one wait slot, so
    a drain waiting on >1 proc fails walrus codegen."""

    def _drain_and_barrier(self, tick_clock, wait_clock):
        import bass_rust
        from concourse.vector_clock import ScopedClock

        gc = tick_clock.global_clock
        for i, v in enumerate(list(gc)):
            if v <= 0:
                continue
            c = bass_rust.VectorClock()
            c.require_at_least(i, v)
            nop = self.nc.sync.nop(hint="preDrain", nofuse=True)
            wait_clock.add_sem_waits(nop.ins, ScopedClock({None: c}))
        drain_inst = self.nc.sync.drain()
        wait_clock.add_sem_waits(
            drain_inst.ins, ScopedClock({None: bass_rust.VectorClock()})
        )
        self.nc.all_engine_barrier()
        assert self.sems is not None
        popped = self.nc._tile_sem_poison_stack.pop()
        assert popped is self._sem_poison
        self.nc.clear_and_free_semaphores(list(self.sems.allocated().values()))
        self.nc.all_engine_barrier()


def pe_touch(nc, ap):
    """1-column bf16 ldweights reading `ap` — engine-level PE instruction
    that absorbs a producer's sync wait into the PE engine clock so that
    following 4-byte matmuls need at most one wait (walrus S3_LW limit)."""
    return nc.tensor.ldweights(ap.bitcast(mybir.dt.bfloat16))


def split_waits(nc):
    """Post-pass: every TPB instruction holds exactly ONE sync-wait slot;
    walrus codegen rejects more. Move extra waits onto inserted same-engine
    nofuse nops placed immediately before the instruction."""
    cnt = 0
    for fn in nc.m.functions:
        for bb in fn.blocks:
            lst = bb.instructions
            i = 0
            while i < len(lst):
                ins = lst[i]
                si = ins.sync_info
                if si is not None and si.on_wait and len(si.on_wait) > 1:
                    waits = list(si.on_wait)
                    for w in waits[:-1]:
                        nop = mybir.InstNoOp(name=f"wsplit_{cnt}", ins=[], outs=[])
                        cnt += 1
                        nop.engine = ins.engine
                        nop.bass_nofuse = True
                        nop.sync_info = mybir.SyncInfo(on_wait=[w], on_update=[])
                        lst.insert(i, nop)
                        i += 1
                    si.on_wait = [waits[-1]]
                i += 1
    return cnt


def build_kernel():
    nc = bass.Bass(trn_type="TRN2")
    q_d = nc.dram_tensor("Q", [HEADS_PER_CORE, S, D], F32, kind="ExternalInput")
    k_d = nc.dram_tensor("K", [HEADS_PER_CORE, S, D], F32, kind="ExternalInput")
    v_d = nc.dram_tensor("V", [HEADS_PER_CORE, S, D], F32, kind="ExternalInput")
    o_d = nc.dram_tensor("O", [HEADS_PER_CORE, S, D], F32, kind="ExternalOutput")

    with SplitDrainTileContext(nc) as tc:
        import contextlib

        with contextlib.ExitStack() as ctx:
            consts = ctx.enter_context(tc.tile_pool(name="consts", bufs=1))
            in_pool = ctx.enter_context(tc.tile_pool(name="in", bufs=2))
            v_pool = ctx.enter_context(tc.tile_pool(name="vp", bufs=2))
            qt_pool = ctx.enter_context(tc.tile_pool(name="qt", bufs=2))
            kt_pool = ctx.enter_context(tc.tile_pool(name="kt", bufs=2))
            pt_pool = ctx.enter_context(tc.tile_pool(name="pt", bufs=6))
            otsb_pool = ctx.enter_context(tc.tile_pool(name="otsb", bufs=2))
            ob_pool = ctx.enter_context(tc.tile_pool(name="ob", bufs=2))
            r_pool = ctx.enter_context(tc.tile_pool(name="recip", bufs=4))

            st_ps = ctx.enter_context(tc.tile_pool(name="stps", bufs=3, space="PSUM"))
            ot_ps = ctx.enter_context(tc.tile_pool(name="otps", bufs=1, space="PSUM"))
            stage_ps = ctx.enter_context(tc.tile_pool(name="stage", bufs=1, space="PSUM"))

            # ---- constants ----
            ident_f = consts.tile([128, 128], F32, tag="ident_f")
            nc.gpsimd.memset(ident_f[:], 0.0)
            nc.gpsimd.affine_select(
                out=ident_f[:], in_=ident_f[:],
                compare_op=mybir.AluOpType.not_equal, fill=1.0, base=0,
                pattern=[[-1, 128]], channel_multiplier=1,
            )
            ident_r = consts.tile([128, 128], F32R, tag="ident_r")
            nc.vector.tensor_copy(ident_r[:], ident_f[:])
            ident_b = consts.tile([128, 128], BF16, tag="ident_b")
            nc.vector.tensor_copy(ident_b[:], ident_f[:])
            # 0/1 causal keep-mask for one diagonal block of P^T [k, q]:
            # keep (1.0) where q >= k i.e. f >= p, zero where f < p.
            tmask = consts.tile([128, 128], BF16, tag="tmask")
            nc.gpsimd.memset(tmask[:], 1.0)
            # keep 1.0 where f - p + 1 > 0 i.e. q >= k; fill 0.0 above diag
            nc.gpsimd.affine_select(
                out=tmask[:], in_=tmask[:],
                compare_op=mybir.AluOpType.is_gt, fill=0.0, base=1,
                pattern=[[1, 128]], channel_multiplier=-1,
            )
            t_if = pe_touch(nc, ident_f[0:1, 0:1])
            t_ir = pe_touch(nc, ident_r[0:1, 0:1])
            t_ib = pe_touch(nc, ident_b[0:1, 0:1])
            # PE warm-up: keep the array busy early so the pstate ramps to
            # full clock while the first loads land.
            warm = stage_ps.tile([128, 512], F32, tag="stage")
            for _ in range(36):
                nc.tensor.matmul(
                    warm[:, 0:256],
                    ident_f[:, 0:64].bitcast(mybir.dt.bfloat16),
                    ident_f[:, 0:128].bitcast(mybir.dt.bfloat16),
                    start=True, stop=True,
                )

            # ---- per-head prep pieces ----
            # Loads are staggered across pair slots (k, then q, then v) so
            # the ~1.4us SWDGE issue instructions on GPSIMD never queue up
            # in front of the diagonal masks, and the PE touches (which
            # carry the DMA-completion waits) are emitted only once the
            # loads have had several pair-slots to land — emitting them
            # with the loads stalls the in-order PE queue for the full DMA
            # latency at every head boundary.
            def emit_load_k(h):
                qn = in_pool.tile([128, NT * 64], BF16, tag="qn")
                kn = in_pool.tile([128, NT * 64], BF16, tag="kn")
                nc.gpsimd.dma_start(
                    kn[:].rearrange("p (t d) -> p t d", d=64),
                    k_d[h].rearrange("(t p) d -> p t d", p=128),
                )
                qt = qt_pool.tile([128, S], BF16, tag="qt")
                kt = kt_pool.tile([128, 8 * 128], BF16, tag="kt")
                return {"qn": qn, "kn": kn, "qt": qt, "kt": kt,
                        "first_tr": None}

            def emit_load_q(hs, h):
                nc.gpsimd.dma_start(
                    hs["qn"][:].rearrange("p (t d) -> p t d", d=64),
                    q_d[h].rearrange("(t p) d -> p t d", p=128),
                )

            def emit_load_v(hs, h):
                vp = v_pool.tile([128, NT * 65], BF16, tag="vp")
                vp3 = vp[:].rearrange("p (t e) -> p t e", e=65)
                nc.gpsimd.dma_start(
                    vp3[:, :, 0:64],
                    v_d[h].rearrange("(t p) d -> p t d", p=128),
                )
                nc.gpsimd.memset(vp3[:, :, 64:65], 1.0)
                hs["vp"] = vp

            def emit_loads(h):
                hs = emit_load_k(h)
                emit_load_q(hs, h)
                emit_load_v(hs, h)
                hs["touch"] = []
                return hs

            def emit_group(hs, g):
                """g 0..3: Q transpose groups, bf16 PE transpose-mode
                (DVE copy + SP row-dup); g 4..5: K^T stacked pairs, bf16
                PE transpose-mode (DVE copy). An XBAR DMA-transpose K was
                tried: each occupies the Sync engine ~1.2us and starves
                the PE at head boundaries — PE transposes are cheaper."""
                if g < 4:
                    stage = stage_ps.tile([128, 512], BF16, tag="stage",
                                          name="stage")
                    for s_i in range(4):
                        b = 4 * g + s_i
                        mm = nc.tensor.transpose(
                            stage[0:64, 128 * s_i:128 * s_i + 128],
                            hs["qn"][:, 64 * b:64 * b + 64],
                            ident_b[0:128, 0:128],
                        )
                        if hs["first_tr"] is None:
                            hs["first_tr"] = mm
                            for t in [t_if, t_ir, t_ib] + hs["touch"]:
                                if t is not None:
                                    add_dep_helper(mm.ins, t.ins, sync=False,
                                                   reason="presync")
                    nc.vector.tensor_copy(
                        hs["qt"][0:64, 512 * g:512 * g + 512],
                        stage[0:64, :],
                    )
                    nc.sync.dma_start(
                        hs["qt"][64:128, 512 * g:512 * g + 512],
                        hs["qt"][0:64, 512 * g:512 * g + 512],
                    )
                else:
                    gg = g - 4
                    stage = stage_ps.tile([128, 512], BF16, tag="stage",
                                          name="stage")
                    for s_i in range(4):
                        t_i = 4 * gg + s_i
                        mm = nc.tensor.transpose(
                            stage[:, 128 * s_i:128 * s_i + 128],
                            hs["kn"][:, 128 * t_i:128 * t_i + 128],
                            ident_b[0:128, 0:128],
                        )
                        if hs["first_tr"] is None:
                            hs["first_tr"] = mm
                            for t in [t_if, t_ir, t_ib] + hs["touch"]:
                                if t is not None:
                                    add_dep_helper(mm.ins, t.ins, sync=False,
                                                   reason="presync")
                    nc.vector.tensor_copy(
                        hs["kt"][:, 512 * gg:512 * gg + 512], stage[:, :]
                    )
                if g == 3:
                    hs["tq1"] = pe_touch(nc, hs["qt"][0:1, 0:1])
                    hs["tk1"] = pe_touch(nc, hs["kt"][0:1, 0:1])

            # ---- pair ops ----
            exp_ctr = [0]

            def emit_qk(hs, h, c, t, first_of_head):
                qt, kt = hs["qt"], hs["kt"]
                j1, j2 = 2 * t, 2 * t + 1
                cA = 128 * j1 - 512 * c
                cB = 128 * j2 - 512 * c
                a1 = max(0, cA)
                a2 = max(0, cB)
                st = st_ps.tile([128, 1024], F32, tag="st")
                mmA = nc.tensor.matmul(
                    st[:, a1:512],
                    kt[0:64, 128 * t:128 * t + 128],
                    qt[0:64, 512 * c + a1:512 * c + 512],
                    start=True, stop=True,
                )
                if first_of_head:
                    for tt in (hs.get("tq1"), hs.get("tk1")):
                        if tt is not None:
                            add_dep_helper(mmA.ins, tt.ins, sync=False,
                                           reason="presync")
                nc.tensor.matmul(
                    st[:, 512 + a2:1024],
                    kt[64:128, 128 * t:128 * t + 128],
                    qt[64:128, 512 * c + a2:512 * c + 512],
                    start=True, stop=True,
                )

                pt = pt_pool.tile([128, 1024], BF16, tag="pt")
                exp_ctr[0] += 1
                # [128, 2, 512] half-major views of P^T / S^T; half 1's
                # [a1:a2) sliver is over-computed as before (harmless).
                pv2 = pt[:].rearrange("p (h x) -> p h x", x=512)
                pi2 = pt[:].bitcast(I16).rearrange("p (h x) -> p h x", x=512)
                sv2 = st[:].rearrange("p (h x) -> p h x", x=512)
                lo = a1
                if lo < EXP_SPLIT - 32:
                    nc.vector.tensor_scalar(
                        pi2[:, :, lo:EXP_SPLIT],
                        sv2[:, :, lo:EXP_SPLIT],
                        FEXP_K1, FEXP_K2,
                        mybir.AluOpType.mult, mybir.AluOpType.add,
                    )
                    m = EXP_SPLIT
                else:
                    m = lo
                nc.scalar.activation(
                    pv2[:, :, m:512], sv2[:, :, m:512],
                    mybir.ActivationFunctionType.Exp, scale=0.125,
                )
                # zero the in-block upper triangles of diagonal tiles: the
                # last two pairs of each chunk hold them, at in-pair col
                # offsets (0, 640) for pair 2c and (256, 896) for pair 2c+1.
                npair = 2 * c + 2
                if t >= npair - 2:
                    off = 0 if t == npair - 2 else 256
                    v8 = pt[:].rearrange("p (i x) -> p i x", x=128)
                    i0 = off // 128
                    dview = v8[:, i0:i0 + 6:5, :]
                    # zero where q < k i.e. f - p + 1 <= 0; runs on GPSIMD
                    # (SBUF-only engine) to keep DVE free for fast-exp.
                    nc.gpsimd.affine_select(
                        out=dview, in_=dview,
                        compare_op=mybir.AluOpType.is_gt, fill=0.0, base=1,
                        pattern=[[0, 2], [1, 128]], channel_multiplier=-1,
                    )
                return {"st": st, "pt": pt}

            def emit_pv(hs, h, c, t, tiles, ot_holder):
                pt, vp = tiles["pt"], hs["vp"]
                npair = 2 * c + 2
                if t == 0:
                    ot_holder["ot"] = ot_ps.tile([65, 512], F32, tag="ot",
                                                 name="ot")
                ot = ot_holder["ot"]
                for half, j in enumerate((2 * t, 2 * t + 1)):
                    vA = max(0, 128 * j - 512 * c)
                    nc.tensor.matmul(
                        ot[:, vA:512],
                        vp[:, 65 * j:65 * j + 65],
                        pt[:, 512 * half + vA:512 * half + 512],
                        start=(t == 0 and half == 0),
                        stop=(t == npair - 1 and half == 1),
                        skip_group_check=True,
                    )

            def emit_out_copy(ot_holder):
                ot = ot_holder["ot"]
                otsb = otsb_pool.tile([65, 512], F32R, tag="otsb")
                nc.vector.tensor_copy(otsb[:], ot[:])
                ot_holder["otsb"] = otsb

            def emit_out(hs, h, c, ot_holder):
                otsb = ot_holder["otsb"]
                oq = stage_ps.tile([128, 384], F32R, tag="stage", name="oq")
                for i in range(4):
                    nc.tensor.transpose(
                        oq[:, 96 * i:96 * i + 96],
                        otsb[0:65, 128 * i:128 * i + 128],
                        ident_r[0:65, 0:96],
                    )
                oq4 = oq[:].bitcast(F32).rearrange("p (i x) -> p i x", x=96)
                rec = r_pool.tile([128, 4], F32, tag="rec")
                nc.vector.reciprocal(rec[:][:, :, None], oq4[:, :, 64:65])
                ob = ob_pool.tile([128, 256], F32, tag="ob")
                nc.vector.tensor_tensor(
                    ob[:].rearrange("p (i x) -> p i x", x=64),
                    oq4[:, :, 0:64],
                    rec[:].broadcast_to([128, 4, 64]),
                    mybir.AluOpType.mult,
                )
                nc.sync.dma_start(
                    o_d[h, 512 * c:512 * c + 512, :].rearrange(
                        "(t p) d -> p t d", p=128),
                    ob[:].rearrange("p (t d) -> p t d", d=64),
                )

            # ---- flat skew-2 pipeline over all (head, chunk, pair) ----
            all_pairs = []
            for h in range(HEADS_PER_CORE):
                for c in range(NCHUNK):
                    for t in range(2 * c + 2):
                        all_pairs.append((h, c, t))

            # K^T groups first so kt is ready when a head's first QK fires;
            # head 0 interleaves its last Q groups into its first pairs.
            PREP_ORDER = [4, 5, 0, 1, 2, 3]
            # need-order K0, Q0, K8, Q1, Q2, Q3 spaced 2 slots apart
            GROUP_SLOTS = {8: 4, 10: 0, 12: 5, 14: 1, 16: 2, 18: 3}
            GROUP_SLOTS_H1 = {4: 4, 6: 0, 8: 5, 10: 1, 12: 2, 14: 3}
            head_state = [None] * HEADS_PER_CORE
            head_state[0] = emit_loads(0)
            # head 1's K/Q loads issue up-front: head 0's early pairs run
            # at the cold 1.2GHz clock, so slot-based prep lead is too
            # short in wall time — without this, head 1's first QKs wait
            # ~3.6us on the row-dup DMAs and re-trigger HAM cold.
            head_state[1] = emit_load_k(1)
            head_state[1]["touch"] = []
            emit_load_q(head_state[1], 1)
            for g in PREP_ORDER[:3]:
                emit_group(head_state[0], g)

            tiles_by_idx = {}
            ot_holders = {}
            out_queue = []  # (due_slot, h, c, holder): PE out-part delayed
            n = len(all_pairs)
            SKEW = 2
            OUT_DELAY = 0

            def run_pv(ip):
                hp, cp, tp = all_pairs[ip]
                key = (hp, cp)
                if key not in ot_holders:
                    ot_holders[key] = {}
                emit_pv(head_state[hp], hp, cp, tp, tiles_by_idx.pop(ip),
                        ot_holders[key])
                if tp == 2 * cp + 1:
                    holder = ot_holders.pop(key)
                    emit_out_copy(holder)
                    out_queue.append([ip + OUT_DELAY, hp, cp, holder])

            def flush_outs(slot):
                while out_queue and out_queue[0][0] <= slot:
                    _, hp, cp, holder = out_queue.pop(0)
                    emit_out(head_state[hp], hp, cp, holder)

            for i, (h, c, t) in enumerate(all_pairs):
                local = i - PAIRS_PER_HEAD * h
                tiles_by_idx[i] = emit_qk(
                    head_state[h], h, c, t, first_of_head=(local == 0))
                if h == 0 and local in (0, 2, 4):
                    emit_group(head_state[0], PREP_ORDER[3 + local // 2])
                if i >= SKEW:
                    run_pv(i - SKEW)
                    flush_outs(i - SKEW)
                if h + 1 < HEADS_PER_CORE:
                    # loads on GPSIMD avoid mask slots (0,1,4,5,10,11,18,19);
                    # groups start once kn/qn have landed so the in-order PE
                    # queue never waits long, and the row-dup DMAs finish
                    # well before the next head's first QK needs them.
                    # Head 1 (cold-clock region): loads were issued up-front,
                    # groups run early at locals 4-9.
                    if h == 0:
                        if local == 2:
                            emit_load_v(head_state[1], 1)
                        if local in GROUP_SLOTS_H1:
                            emit_group(head_state[1], GROUP_SLOTS_H1[local])
                    else:
                        if local == 2:
                            head_state[h + 1] = emit_load_k(h + 1)
                            head_state[h + 1]["touch"] = []
                        elif local == 3:
                            emit_load_q(head_state[h + 1], h + 1)
                        elif local == 6:
                            emit_load_v(head_state[h + 1], h + 1)
                        # groups spaced 2 slots apart in deadline order: the
                        # bufs=1 stage bank serializes transpose(g+1) behind
                        # group g's DVE copy, and 1-slot spacing made the
                        # in-order PE queue eat that wait. Q-group g's qt
                        # columns are first read at chunk g of the next head
                        # (locals 20/22/26/32); K8-15 at local 26.
                        if local in GROUP_SLOTS:
                            emit_group(head_state[h + 1],
                                       GROUP_SLOTS[local])
            for ip in range(n - SKEW, n):
                run_pv(ip)
                flush_outs(ip)
            flush_outs(10 ** 9)

    split_waits(nc)
    return nc


_CACHED = {}


def kernel(Q: np.ndarray, K: np.ndarray, V: np.ndarray) -> np.ndarray:
    res = _run(Q, K, V, trace=False)
    return res[0]


def _run(Q, K, V, trace=False):
    Qf = np.ascontiguousarray(Q.reshape(B * H, S, D), dtype=np.float32)
    Kf = np.ascontiguousarray(K.reshape(B * H, S, D), dtype=np.float32)
    Vf = np.ascontiguousarray(V.reshape(B * H, S, D), dtype=np.float32)

    in_maps = []
    for c in range(N_CORES):
        sl = slice(c * HEADS_PER_CORE, (c + 1) * HEADS_PER_CORE)
        in_maps.append({
            "Q": np.ascontiguousarray(Qf[sl]),
            "K": np.ascontiguousarray(Kf[sl]),
            "V": np.ascontiguousarray(Vf[sl]),
        })

    if "nc" not in _CACHED:
        _CACHED["nc"] = build_kernel()
    nc = _CACHED["nc"]

    res = run_bass_kernel_spmd(
        nc, in_maps, core_ids=list(range(N_CORES)), trace=trace
    )
    out = np.empty((B * H, S, D), dtype=np.float32)
    for c in range(N_CORES):
        out[c * HEADS_PER_CORE:(c + 1) * HEADS_PER_CORE] = res.results[c]["O"]
    return out.reshape(B, H, S, D), res

